# revision 16
# baseline (speedup 1.0000x reference)
"""Trainium2 Bass kernel for nn_Attention (LayerNorm + L2-normalized-QK attention
with null-kv slot + output projection), SPMD across 8 NeuronCores.

Sharding: core c = (batch b = c//2, query-half hi = c%2). Each core computes the
full kv (2048 tokens) of its batch and attention outputs for its 1024-query
half. Softmax over kv is permutation invariant, so for hi=1 we feed x with the
two sequence halves swapped — every core runs the identical SPMD program with
its queries in rows 0:1024. The final output is a pure concatenation of the
per-core results (no collectives).

Key structural ideas (v1 rewrite):
  - LayerNorm is folded into the projections: project raw x^T (host passes a
    pre-transposed bf16 copy), then subtract the rank-1 correction
    mu[t] * colsum(gamma*W) via one extra K=2 matmul accumulated into the same
    PSUM tile (row0 = mu, row1 = ones for the beta term). The per-token rstd
    cancels inside the q/k L2 norms and is applied to v as a per-partition
    scalar. This removes the serial LN -> transpose -> proj phase entirely.
  - S is computed transposed (S^T [kv, q]); q,k are L2-normalized so
    |8*q.k| <= 8 and exp() cannot overflow (no row-max pass).
  - The two heads of a pair run their S matmuls concurrently via row-group
    tiling (K=64 each at tile_position (0,0)/(64,0)).
  - PV uses V' = [V | 1] (M=65) so the softmax denominator falls out of the
    same matmul chain, and the output lands directly in A^T layout for the
    output projection.
  - rsqrt = exp(-0.5*ln(x)): the only ACT table set used in the whole program
    is natural_log_exp_and_others (square/ln/exp all live there -> one load).
  - q_scale*k_scale is folded into kT as a per-partition scale post-transpose.
  - Weights are pre-folded with gamma, cast to bf16, and reordered on host;
    null-kv tensors are fully precomputed on host in float64.
"""

import numpy as np

B = 4
N = 2048
DIM = 1024
HEADS = 16
DH = 64
INNER = HEADS * DH
NQ = 1024  # queries per core
SCALE = 8.0
LN_EPS = 1e-5

_CACHE = {}


def _build_program(beta_zero: bool, qk_ones: bool):
    from contextlib import ExitStack

    import concourse.bacc as bacc
    import concourse.tile as tile
    from concourse import mybir

    f32 = mybir.dt.float32
    bf16 = mybir.dt.bfloat16
    AF = mybir.ActivationFunctionType
    OP = mybir.AluOpType
    AX = mybir.AxisListType

    NT = N // 128          # 16 token tiles
    NTQ = NQ // 128        # 8 query token tiles
    NCD = DIM // 128       # 8 dim chunks
    HP = HEADS // 2        # 8 head pairs

    nc = bacc.Bacc("TRN2", target_bir_lowering=False, debug=False)

    x = nc.declare_dram_parameter("x", [N, DIM], f32, isOutput=False)
    xT = nc.declare_dram_parameter("xT", [DIM, N], bf16, isOutput=False)
    Wk = nc.declare_dram_parameter("Wk", [DIM, INNER], bf16, isOutput=False)
    Wq = nc.declare_dram_parameter("Wq", [DIM, INNER], bf16, isOutput=False)
    Wv = nc.declare_dram_parameter("Wv", [DIM, INNER], bf16, isOutput=False)
    Wo = nc.declare_dram_parameter("Wo", [INNER, DIM], bf16, isOutput=False)
    # correction row: -colsum(gamma*W) (computed from the bf16 weights)
    wbbk = nc.declare_dram_parameter("wbbk", [1, INNER], bf16, isOutput=False)
    wbbq = nc.declare_dram_parameter("wbbq", [1, INNER], bf16, isOutput=False)
    wbbv = nc.declare_dram_parameter("wbbv", [1, INNER], bf16, isOutput=False)
    if not beta_zero:
        bbk = nc.declare_dram_parameter("bbk", [INNER], f32, isOutput=False)
        bbq = nc.declare_dram_parameter("bbq", [INNER], f32, isOutput=False)
        bbv = nc.declare_dram_parameter("bbv", [INNER], f32, isOutput=False)
    nkn_p = nc.declare_dram_parameter("nkn_bd", [128, HEADS], bf16, isOutput=False)
    nv_p = nc.declare_dram_parameter("nv_bd2", [2, HEADS, DH + 1], bf16, isOutput=False)
    # per-partition qs*ks pattern for kT scaling ([d0..63, d0..63])
    qkc = nc.declare_dram_parameter("qkcol", [128, 1], f32, isOutput=False)
    out = nc.declare_dram_parameter("out", [NQ, DIM], f32, isOutput=True)

    # internal DRAM
    kn_d = nc.dram_tensor("kn_d", [N, INNER], bf16)
    qn_d = nc.dram_tensor("qn_d", [NQ, INNER], bf16)
    mu_d = nc.dram_tensor("mu_d", [NT, 128], bf16)
    rcp_d = nc.dram_tensor("rcp_d", [HEADS, 2, DH, 8], f32)
    den_d = nc.dram_tensor("den_d", [HEADS, 2, DH, 8], f32)

    with tile.TileContext(nc) as tc, ExitStack() as ctx:
        persist = ctx.enter_context(tc.tile_pool(name="persist", bufs=1))

        # ---------------- persistent SBUF ----------------
        kT = persist.tile([128, HP, N], bf16, tag="kT")
        qT = persist.tile([128, HP, NQ], bf16, tag="qT")
        vsb = persist.tile([128, NT, HEADS, DH + 1], bf16, tag="v")
        AT = persist.tile([128, NCD, NQ], bf16, tag="AT")

        nkn_sb = persist.tile([128, HEADS], bf16)
        nc.sync.dma_start(out=nkn_sb, in_=nkn_p[:, :])
        nv_sb = persist.tile([2, HEADS, DH + 1], bf16)
        nc.sync.dma_start(out=nv_sb, in_=nv_p[:, :, :])
        wbbk_sb = persist.tile([1, INNER], bf16)
        nc.sync.dma_start(out=wbbk_sb, in_=wbbk[:, :])
        wbbq_sb = persist.tile([1, INNER], bf16)
        nc.sync.dma_start(out=wbbq_sb, in_=wbbq[:, :])
        wbbv_sb = persist.tile([1, INNER], bf16)
        nc.sync.dma_start(out=wbbv_sb, in_=wbbv[:, :])
        qk_sb = persist.tile([128, 1], f32)
        nc.sync.dma_start(out=qk_sb, in_=qkc[:, :])
        if not beta_zero:
            bbk_b = persist.tile([128, INNER], f32)
            nc.gpsimd.dma_start(out=bbk_b, in_=bbk.ap().partition_broadcast(128))
            bbq_b = persist.tile([128, INNER], f32)
            nc.gpsimd.dma_start(out=bbq_b, in_=bbq.ap().partition_broadcast(128))
            bbv_b = persist.tile([128, INNER], f32)
            nc.gpsimd.dma_start(out=bbv_b, in_=bbv.ap().partition_broadcast(128))
        else:
            bbk_b = bbq_b = bbv_b = None

        mu1 = persist.tile([1, N], bf16, tag="mu1")

        mv = persist.tile([128, NT, 2], f32)
        rst = persist.tile([128, NT], f32)
        eps_t = persist.tile([128, 1], f32)
        nc.vector.memset(eps_t, LN_EPS)
        eps30 = persist.tile([128, 1], f32)
        nc.vector.memset(eps30, 1e-30)

        nc.vector.memset(vsb[:, :, :, DH : DH + 1], 1.0)  # ones column of V'

        # ---------------- projections (LN folded in) ----------------
        with (
            tc.tile_pool(name="pxw", bufs=1) as pxw,
            tc.tile_pool(name="pw", bufs=2) as pw,
            tc.tile_pool(name="pxs", bufs=3) as pxs,
            tc.tile_pool(name="pst", bufs=4) as pst,
            tc.tile_pool(name="ppj", bufs=4, space="PSUM") as ppj,
            tc.tile_pool(name="pnrm", bufs=3) as pnrm,
            tc.tile_pool(name="pkn", bufs=3) as pkn,
        ):
            # stats first: the mu round trip is the critical path to the first
            # finished projection tile. x tiles on the sync ring, xTb on the
            # scalar ring, weights on the gpsimd (SWDGE) ring — all parallel.
            for tt in range(NT):
                r0 = tt * 128
                xt = pxs.tile([128, DIM], f32, tag="xt")
                nc.sync.dma_start(out=xt, in_=x[r0 : r0 + 128, :])
                stats = pst.tile([128, 2, 6], f32, tag="stats")
                nc.vector.bn_stats(out=stats[:, 0, :], in_=xt[:, 0:512])
                nc.vector.bn_stats(out=stats[:, 1, :], in_=xt[:, 512:1024])
                nc.vector.bn_aggr(out=mv[:, tt, :], in_=stats)

            xTb = pxw.tile([128, NCD, N], bf16, tag="xTb")
            for c in range(NCD):
                nc.scalar.dma_start(out=xTb[:, c, :], in_=xT[c * 128 : (c + 1) * 128, :])

            wk_sb = pw.tile([128, NCD, INNER], bf16, tag="W")
            for c in range(NCD):
                nc.gpsimd.dma_start(
                    out=wk_sb[:, c, :], in_=Wk[c * 128 : (c + 1) * 128, :]
                )
            wq_sb = pw.tile([128, NCD, INNER], bf16, tag="W")
            for c in range(NCD):
                nc.gpsimd.dma_start(
                    out=wq_sb[:, c, :], in_=Wq[c * 128 : (c + 1) * 128, :]
                )
            # rstd (batched): exp(-0.5*ln(var+eps))
            nc.scalar.activation(out=rst, in_=mv[:, :, 1], func=AF.Ln, bias=eps_t)
            nc.scalar.activation(out=rst, in_=rst, func=AF.Exp, scale=-0.5)
            # mu -> [1, N] row via DRAM round trip
            mub = pnrm.tile([128, NT], bf16, tag="mub")
            nc.vector.tensor_copy(out=mub, in_=mv[:, :, 0])
            nc.sync.dma_start(out=mu_d.ap().rearrange("t p -> p t"), in_=mub)
            nc.sync.dma_start(
                out=mu1[0:1, :],
                in_=mu_d.ap().rearrange("t p -> (t p)").partition_broadcast(1),
            )

            def proj_norm_tiles(w_sb, wbb_sb, bb_b, nd, ntiles):
                """k/q projection + per-tile l2norm scale -> nd DRAM (bf16)."""
                for half in range(2):
                    cs = half * 512
                    for tt in range(ntiles):
                        r0 = tt * 128
                        kp = ppj.tile([128, 512], f32, tag="pj")
                        for c in range(NCD):
                            nc.tensor.matmul(
                                kp,
                                lhsT=xTb[:, c, r0 : r0 + 128],
                                rhs=w_sb[:, c, cs : cs + 512],
                                start=(c == 0),
                                stop=False,
                            )
                        nc.tensor.matmul(
                            kp,
                            lhsT=mu1[:, r0 : r0 + 128],
                            rhs=wbb_sb[:, cs : cs + 512],
                            start=False,
                            stop=True,
                        )
                        # copy to bf16 (releases PSUM), square+reduce on DVE so
                        # the ACT engine only ever runs Ln/Exp (one table set)
                        kcb = pnrm.tile([128, 512], bf16, tag="kcb")
                        if beta_zero:
                            nc.vector.tensor_copy(out=kcb, in_=kp)
                        else:
                            # general path: k = rstd*(kp) + beta@W (bcast row)
                            kf = pnrm.tile([128, 512], f32, tag="kf")
                            nc.vector.tensor_scalar_mul(
                                out=kf, in0=kp, scalar1=rst[:, tt : tt + 1]
                            )
                            nc.vector.tensor_tensor(
                                out=kcb, in0=kf, in1=bb_b[:, cs : cs + 512], op=OP.add
                            )
                        sq = pnrm.tile([128, 512], bf16, tag="sq")
                        nc.vector.tensor_tensor(out=sq, in0=kcb, in1=kcb, op=OP.mult)
                        s2 = pnrm.tile([128, 8], f32, tag="s2")
                        nc.vector.tensor_reduce(
                            out=s2,
                            in_=sq.rearrange("p (g d) -> p g d", g=8),
                            axis=AX.X,
                            op=OP.add,
                        )
                        nc.scalar.activation(out=s2, in_=s2, func=AF.Ln, bias=eps30)
                        nc.scalar.activation(out=s2, in_=s2, func=AF.Exp, scale=-0.5)
                        nc.vector.tensor_scalar_min(out=s2, in0=s2, scalar1=1e12)
                        rex = pnrm.tile([128, 8, DH], bf16, tag="rex")
                        nc.vector.tensor_copy(
                            out=rex, in_=s2.broadcast_to([128, 8, DH])
                        )
                        kn = pkn.tile([128, 512], bf16, tag="kn")
                        nc.vector.tensor_tensor(
                            out=kn,
                            in0=kcb,
                            in1=rex.rearrange("p g d -> p (g d)"),
                            op=OP.mult,
                        )
                        nc.sync.dma_start(
                            out=nd[r0 : r0 + 128, cs : cs + 512], in_=kn
                        )

            proj_norm_tiles(wk_sb, wbbk_sb, bbk_b, kn_d, NT)
            # kT transposes (scalar HWDGE ring) + qs*ks per-partition fold
            for p in range(HP):
                nc.scalar.dma_start(
                    out=kT[:, p, :], in_=kn_d[:, p * 128 : (p + 1) * 128],
                    transpose=True,
                )
                if not qk_ones:
                    nc.vector.tensor_scalar_mul(
                        out=kT[:, p, :], in0=kT[:, p, :], scalar1=qk_sb
                    )

            proj_norm_tiles(wq_sb, wbbq_sb, bbq_b, qn_d, NTQ)
            for p in range(HP):
                nc.scalar.dma_start(
                    out=qT[:, p, :], in_=qn_d[:, p * 128 : (p + 1) * 128],
                    transpose=True,
                )

            # ---- v projection -> V' natural layout
            wv_sb = pw.tile([128, NCD, INNER], bf16, tag="W")
            for c in range(NCD):
                nc.gpsimd.dma_start(
                    out=wv_sb[:, c, :], in_=Wv[c * 128 : (c + 1) * 128, :]
                )
            for half in range(2):
                cs = half * 512
                for tt in range(NT):
                    r0 = tt * 128
                    vp = ppj.tile([128, 512], f32, tag="pj")
                    for c in range(NCD):
                        nc.tensor.matmul(
                            vp,
                            lhsT=xTb[:, c, r0 : r0 + 128],
                            rhs=wv_sb[:, c, cs : cs + 512],
                            start=(c == 0),
                            stop=False,
                        )
                    nc.tensor.matmul(
                        vp,
                        lhsT=mu1[:, r0 : r0 + 128],
                        rhs=wbbv_sb[:, cs : cs + 512],
                        start=False,
                        stop=True,
                    )
                    if beta_zero:
                        nc.vector.tensor_scalar_mul(
                            out=vsb[:, tt, half * 8 : (half + 1) * 8, 0:DH],
                            in0=vp.rearrange("p (g d) -> p g d", g=8),
                            scalar1=rst[:, tt : tt + 1],
                        )
                    else:
                        vf = pnrm.tile([128, 512], f32, tag="kf")
                        nc.vector.tensor_scalar_mul(
                            out=vf, in0=vp, scalar1=rst[:, tt : tt + 1]
                        )
                        nc.vector.tensor_tensor(
                            out=vsb[:, tt, half * 8 : (half + 1) * 8, 0:DH],
                            in0=vf.rearrange("p (g d) -> p g d", g=8),
                            in1=bbv_b[:, cs : cs + 512].rearrange(
                                "p (g d) -> p g d", g=8
                            ),
                            op=OP.add,
                        )

        # ---------------- attention + output projection ----------------
        QB = NQ // 512  # 2 query blocks of 512
        with (
            tc.tile_pool(name="pwo", bufs=1) as pwo,
            tc.tile_pool(name="pstt", bufs=2, space="PSUM") as pstt,
            tc.tile_pool(name="pot", bufs=3, space="PSUM") as pot,
            tc.tile_pool(name="po", bufs=1, space="PSUM") as po,
            tc.tile_pool(name="ppt", bufs=3) as ppt,
            tc.tile_pool(name="pptn", bufs=2) as pptn,
            tc.tile_pool(name="prec", bufs=2) as prec,
            tc.tile_pool(name="pbsc", bufs=2) as pbsc,
            tc.tile_pool(name="pob", bufs=3) as pob,
        ):
            wo_sb = pwo.tile([128, NCD, INNER], bf16, tag="Wo")
            for c in range(NCD):
                nc.gpsimd.dma_start(
                    out=wo_sb[:, c, :], in_=Wo[c * 128 : (c + 1) * 128, :]
                )

            def oproj_group(half, tt):
                r0 = tt * 128
                cs = half * 512
                op_ = po.tile([128, 512], f32, tag="op")
                for c in range(NCD):
                    nc.tensor.matmul(
                        op_,
                        lhsT=AT[:, c, r0 : r0 + 128],
                        rhs=wo_sb[:, c, cs : cs + 512],
                        start=(c == 0),
                        stop=(c == NCD - 1),
                    )
                ob = pob.tile([128, 512], f32, tag="ob")
                nc.vector.tensor_copy(out=ob, in_=op_)
                nc.sync.dma_start(out=out[r0 : r0 + 128, cs : cs + 512], in_=ob)

            for hp in range(HP):
                hA, hB = 2 * hp, 2 * hp + 1
                for qb in range(QB):
                    q0 = qb * 512
                    last_pair = hp == HP - 1 and qb == QB - 1
                    # null scores for both heads
                    st_n = pstt.tile([128, 2, 512], f32, tag="st")
                    null_ps = st_n[0:2, 0, :]
                    nc.tensor.matmul(
                        null_ps,
                        lhsT=nkn_sb[:, hA : hA + 2],
                        rhs=qT[:, hp, q0 : q0 + 512],
                        start=True,
                        stop=True,
                    )
                    pTn = pptn.tile([2, 512], bf16)
                    nc.scalar.activation(out=pTn, in_=null_ps, func=AF.Exp, scale=SCALE)

                    otA = pot.tile([DH + 1, 512], f32, tag="ot")
                    otB = pot.tile([DH + 1, 512], f32, tag="ot")

                    for c in range(NT):
                        st = pstt.tile([128, 2, 512], f32, tag="st")
                        for si, rh in ((0, 0), (1, 1)):
                            nc.tensor.matmul(
                                st[:, si, :],
                                lhsT=kT[
                                    rh * DH : (rh + 1) * DH, hp, c * 128 : (c + 1) * 128
                                ],
                                rhs=qT[rh * DH : (rh + 1) * DH, hp, q0 : q0 + 512],
                                start=True,
                                stop=True,
                                tile_position=(rh * DH, 0),
                            )
                        pt = ppt.tile([128, 2, 512], bf16)
                        nc.scalar.activation(out=pt, in_=st, func=AF.Exp, scale=SCALE)
                        for ot, si, h in ((otA, 0, hA), (otB, 1, hB)):
                            nc.tensor.matmul(
                                ot,
                                lhsT=vsb[:, c, h, :],
                                rhs=pt[:, si, :],
                                start=(c == 0),
                                stop=False,
                            )
                        # interleave o-proj for the first query block into the
                        # last attention pair's PE slack
                        if last_pair and c % 2 == 1:
                            g = c // 2
                            oproj_group(g // 4, g % 4)
                    nc.tensor.matmul(
                        otA, lhsT=nv_sb[:, hA, :], rhs=pTn, start=False, stop=True
                    )
                    nc.tensor.matmul(
                        otB, lhsT=nv_sb[:, hB, :], rhs=pTn, start=False, stop=True
                    )
                    # divide by denominator (row DH) and write A^T
                    for h, ot in ((hA, otA), (hB, otB)):
                        den_s = prec.tile([1, 512], f32, tag="dens")
                        nc.vector.tensor_copy(out=den_s, in_=ot[DH : DH + 1, :])
                        nc.sync.dma_start(
                            out=den_d[h, qb]
                            .rearrange("a b -> (a b)")
                            .partition_broadcast(1),
                            in_=den_s,
                        )
                        dd = prec.tile([DH, 8], f32, tag="dd")
                        nc.sync.dma_start(out=dd, in_=den_d[h, qb])
                        rr = prec.tile([DH, 8], f32, tag="rr")
                        nc.vector.reciprocal(rr, dd)
                        nc.sync.dma_start(out=rcp_d[h, qb], in_=rr)
                        rcs = pbsc.tile([DH, 512], f32, tag="bcs")
                        nc.sync.dma_start(
                            out=rcs,
                            in_=rcp_d[h, qb]
                            .rearrange("a b -> (a b)")
                            .partition_broadcast(DH),
                        )
                        po_ = (h % 2) * DH
                        nc.vector.tensor_tensor(
                            out=AT[po_ : po_ + DH, hp, q0 : q0 + 512],
                            in0=ot[0:DH, :],
                            in1=rcs,
                            op=OP.mult,
                        )

            # remaining o-proj groups (query block 1 tokens)
            for half in range(2):
                for tt in range(4, NTQ):
                    oproj_group(half, tt)

    nc.compile()
    return nc


def _get_program(beta_zero: bool = True, qk_ones: bool = True):
    key = ("nc", beta_zero, qk_ones)
    if key not in _CACHE:
        _CACHE[key] = _build_program(beta_zero, qk_ones)
    return _CACHE[key]


def _prep(inputs) -> tuple[list[dict], bool]:
    """Host-side prep: shard + precompute per-core parameter maps."""
    import ml_dtypes

    bf16 = ml_dtypes.bfloat16

    x = np.asarray(inputs["x"], dtype=np.float32)
    gamma = np.asarray(inputs["gamma"], dtype=np.float64)
    beta = np.asarray(inputs["beta"], dtype=np.float64)
    null_kv = np.asarray(inputs["null_kv"], dtype=np.float64)
    Wq = np.asarray(inputs["Wq"], dtype=np.float64)
    Wkv = np.asarray(inputs["Wkv"], dtype=np.float64)
    qs = np.asarray(inputs["q_scale"], dtype=np.float64)
    ks = np.asarray(inputs["k_scale"], dtype=np.float64)
    Wo = np.asarray(inputs["Wo"], dtype=np.float64)

    beta_zero = not np.any(beta)

    Wk = Wkv[:, :INNER]
    Wv = Wkv[:, INNER:]

    def prep_w(W):
        Wg = (gamma[:, None] * W).astype(bf16)
        wbar = Wg.astype(np.float64).sum(axis=0)
        wbb = np.ascontiguousarray((-wbar).astype(bf16).reshape(1, INNER))
        bb = np.ascontiguousarray((beta @ W).astype(np.float32))
        return np.ascontiguousarray(Wg), wbb, bb

    Wk_b, wbbk, bbk = prep_w(Wk)
    Wq_b, wbbq, bbq = prep_w(Wq)
    Wv_b, wbbv, bbv = prep_w(Wv)
    Wo_b = np.ascontiguousarray(Wo.astype(bf16))

    # null-kv prep (float64): nkn = l2norm(nk) * (qs*ks), block-diagonal
    nk = null_kv[0, :, 0, :]  # [H, DH]
    nv = null_kv[1, :, 0, :]
    nrm = np.sqrt((nk * nk).sum(-1, keepdims=True))
    nkn = nk / np.maximum(nrm, 1e-12) * (qs * ks)[None, :]
    nkn_bd = np.zeros((128, HEADS), dtype=np.float64)
    for h in range(HEADS):
        if h % 2 == 0:
            nkn_bd[0:DH, h] = nkn[h]
        else:
            nkn_bd[DH:128, h] = nkn[h]
    nv_bd2 = np.zeros((2, HEADS, DH + 1), dtype=np.float64)
    for h in range(HEADS):
        nv_bd2[h % 2, h, 0:DH] = nv[h]
        nv_bd2[h % 2, h, DH] = 1.0
    nkn_bd = nkn_bd.astype(bf16)
    nv_bd2 = nv_bd2.astype(bf16)

    qkcol = np.tile((qs * ks).astype(np.float32), 2).reshape(128, 1)
    qkcol = np.ascontiguousarray(qkcol)
    qk_ones = bool(np.all(qs * ks == 1.0))

    in_maps = []
    for b in range(B):
        for hi in range(2):
            xb = x[b]
            if hi == 1:
                xb = np.concatenate([xb[NQ:], xb[:NQ]], axis=0)
            xb = np.ascontiguousarray(xb)
            xTb = np.ascontiguousarray(xb.T.astype(bf16))
            m = {
                "x": xb,
                "xT": xTb,
                "Wk": Wk_b,
                "Wq": Wq_b,
                "Wv": Wv_b,
                "Wo": Wo_b,
                "wbbk": wbbk,
                "wbbq": wbbq,
                "wbbv": wbbv,
                "nkn_bd": nkn_bd,
                "nv_bd2": nv_bd2,
                "qkcol": qkcol,
            }
            if not beta_zero:
                m.update({"bbk": bbk, "bbq": bbq, "bbv": bbv})
            in_maps.append(m)
    return in_maps, (beta_zero, qk_ones)


def kernel(**inputs) -> np.ndarray:
    from concourse.bass_utils import run_bass_kernel_spmd

    in_maps, (beta_zero, qk_ones) = _prep(inputs)
    nc = _get_program(beta_zero=beta_zero, qk_ones=qk_ones)

    res = run_bass_kernel_spmd(nc, in_maps, list(range(8)))

    full = np.empty((B, N, DIM), dtype=np.float32)
    for c in range(8):
        b, hi = divmod(c, 2)
        full[b, hi * NQ : (hi + 1) * NQ] = res.results[c]["out"]
    return full


# revision 20
# speedup vs baseline: 1.1422x; 1.1422x over previous
"""Trainium2 Bass kernel for nn_Attention (LayerNorm + L2-normalized-QK attention
with null-kv slot + output projection), SPMD across 8 NeuronCores.

Sharding: core c = (batch b = c//2, query-half hi = c%2). Each core computes the
full kv (2048 tokens) of its batch and attention outputs for its 1024-query
half. Softmax over kv is permutation invariant, so for hi=1 we feed x with the
two sequence halves swapped — every core runs the identical SPMD program with
its queries in rows 0:1024. The final output is a pure concatenation of the
per-core results (no collectives).

Key structural ideas (v1 rewrite):
  - LayerNorm is folded into the projections: project raw x^T (host passes a
    pre-transposed bf16 copy), then subtract the rank-1 correction
    mu[t] * colsum(gamma*W) via one extra K=2 matmul accumulated into the same
    PSUM tile (row0 = mu, row1 = ones for the beta term). The per-token rstd
    cancels inside the q/k L2 norms and is applied to v as a per-partition
    scalar. This removes the serial LN -> transpose -> proj phase entirely.
  - S is computed transposed (S^T [kv, q]); q,k are L2-normalized so
    |8*q.k| <= 8 and exp() cannot overflow (no row-max pass).
  - The two heads of a pair run their S matmuls concurrently via row-group
    tiling (K=64 each at tile_position (0,0)/(64,0)).
  - PV uses V' = [V | 1] (M=65) so the softmax denominator falls out of the
    same matmul chain, and the output lands directly in A^T layout for the
    output projection.
  - rsqrt = exp(-0.5*ln(x)): the only ACT table set used in the whole program
    is natural_log_exp_and_others (square/ln/exp all live there -> one load).
  - q_scale*k_scale is folded into kT as a per-partition scale post-transpose.
  - Weights are pre-folded with gamma, cast to bf16, and reordered on host;
    null-kv tensors are fully precomputed on host in float64.
"""

import numpy as np

B = 4
N = 2048
DIM = 1024
HEADS = 16
DH = 64
INNER = HEADS * DH
NQ = 1024  # queries per core
SCALE = 8.0
LN_EPS = 1e-5

_CACHE = {}


def _build_program(beta_zero: bool, qk_ones: bool):
    from contextlib import ExitStack

    import concourse.bacc as bacc
    import concourse.tile as tile
    from concourse import mybir

    f32 = mybir.dt.float32
    bf16 = mybir.dt.bfloat16
    AF = mybir.ActivationFunctionType
    OP = mybir.AluOpType
    AX = mybir.AxisListType

    NT = N // 128          # 16 token tiles
    NTQ = NQ // 128        # 8 query token tiles
    NCD = DIM // 128       # 8 dim chunks
    HP = HEADS // 2        # 8 head pairs

    nc = bacc.Bacc("TRN2", target_bir_lowering=False, debug=False)

    x = nc.declare_dram_parameter("x", [N, DIM], f32, isOutput=False)
    xT = nc.declare_dram_parameter("xT", [DIM, N], bf16, isOutput=False)
    Wk = nc.declare_dram_parameter("Wk", [DIM, INNER], bf16, isOutput=False)
    Wq = nc.declare_dram_parameter("Wq", [DIM, INNER], bf16, isOutput=False)
    Wv = nc.declare_dram_parameter("Wv", [DIM, INNER], bf16, isOutput=False)
    Wo = nc.declare_dram_parameter("Wo", [INNER, DIM], bf16, isOutput=False)
    # correction row: -colsum(gamma*W) (computed from the bf16 weights)
    wbbk = nc.declare_dram_parameter("wbbk", [1, INNER], bf16, isOutput=False)
    wbbq = nc.declare_dram_parameter("wbbq", [1, INNER], bf16, isOutput=False)
    wbbv = nc.declare_dram_parameter("wbbv", [1, INNER], bf16, isOutput=False)
    if not beta_zero:
        bbk = nc.declare_dram_parameter("bbk", [INNER], f32, isOutput=False)
        bbq = nc.declare_dram_parameter("bbq", [INNER], f32, isOutput=False)
        bbv = nc.declare_dram_parameter("bbv", [INNER], f32, isOutput=False)
    nkn_p = nc.declare_dram_parameter("nkn_bd", [128, HEADS], bf16, isOutput=False)
    nv_p = nc.declare_dram_parameter("nv_bd2", [2, HEADS, DH + 1], bf16, isOutput=False)
    # per-partition qs*ks pattern for kT scaling ([d0..63, d0..63])
    qkc = nc.declare_dram_parameter("qkcol", [128, 1], f32, isOutput=False)
    out = nc.declare_dram_parameter("out", [NQ, DIM], f32, isOutput=True)

    # internal DRAM
    kn_d = nc.dram_tensor("kn_d", [N, INNER], bf16)
    qn_d = nc.dram_tensor("qn_d", [NQ, INNER], bf16)
    mu_d = nc.dram_tensor("mu_d", [NT, 128], bf16)
    rcp_d = nc.dram_tensor("rcp_d", [HEADS, 2, DH, 8], f32)
    den_d = nc.dram_tensor("den_d", [HEADS, 2, DH, 8], f32)

    with tile.TileContext(nc) as tc, ExitStack() as ctx:
        persist = ctx.enter_context(tc.tile_pool(name="persist", bufs=1))

        # ---------------- persistent SBUF ----------------
        kT = persist.tile([128, HP, N], bf16, tag="kT")
        qT = persist.tile([128, HP, NQ], bf16, tag="qT")
        vsb = persist.tile([128, NT, HEADS, DH + 1], bf16, tag="v")
        AT = persist.tile([128, NCD, NQ], bf16, tag="AT")

        nkn_sb = persist.tile([128, HEADS], bf16)
        nc.sync.dma_start(out=nkn_sb, in_=nkn_p[:, :])
        nv_sb = persist.tile([2, HEADS, DH + 1], bf16)
        nc.sync.dma_start(out=nv_sb, in_=nv_p[:, :, :])
        wbbk_sb = persist.tile([1, INNER], bf16)
        nc.sync.dma_start(out=wbbk_sb, in_=wbbk[:, :])
        wbbq_sb = persist.tile([1, INNER], bf16)
        nc.sync.dma_start(out=wbbq_sb, in_=wbbq[:, :])
        wbbv_sb = persist.tile([1, INNER], bf16)
        nc.sync.dma_start(out=wbbv_sb, in_=wbbv[:, :])
        qk_sb = persist.tile([128, 1], f32)
        nc.sync.dma_start(out=qk_sb, in_=qkc[:, :])
        if not beta_zero:
            bbk_b = persist.tile([128, INNER], f32)
            nc.gpsimd.dma_start(out=bbk_b, in_=bbk.ap().partition_broadcast(128))
            bbq_b = persist.tile([128, INNER], f32)
            nc.gpsimd.dma_start(out=bbq_b, in_=bbq.ap().partition_broadcast(128))
            bbv_b = persist.tile([128, INNER], f32)
            nc.gpsimd.dma_start(out=bbv_b, in_=bbv.ap().partition_broadcast(128))
        else:
            bbk_b = bbq_b = bbv_b = None

        mu1 = persist.tile([1, N], bf16, tag="mu1")

        mv = persist.tile([128, NT, 2], f32)
        rst = persist.tile([128, NT], f32)
        eps_t = persist.tile([128, 1], f32)
        nc.vector.memset(eps_t, LN_EPS)
        eps30 = persist.tile([128, 1], f32)
        nc.vector.memset(eps30, 1e-30)

        nc.vector.memset(vsb[:, :, :, DH : DH + 1], 1.0)  # ones column of V'

        # ---------------- projections (LN folded in) ----------------
        with (
            tc.tile_pool(name="pxw", bufs=1) as pxw,
            tc.tile_pool(name="pw", bufs=2) as pw,
            tc.tile_pool(name="pxs", bufs=3) as pxs,
            tc.tile_pool(name="pst", bufs=4) as pst,
            tc.tile_pool(name="ppj", bufs=4, space="PSUM") as ppj,
            tc.tile_pool(name="pnrm", bufs=3) as pnrm,
            tc.tile_pool(name="pkn", bufs=3) as pkn,
            tc.tile_pool(name="pstash", bufs=1) as pstash,
        ):
            # stats first: the mu round trip is the critical path to the first
            # finished projection tile. x tiles on the sync ring, xTb on the
            # scalar ring, weights on the gpsimd (SWDGE) ring — all parallel.
            for tt in range(NT):
                r0 = tt * 128
                xt = pxs.tile([128, DIM], f32, tag="xt")
                nc.sync.dma_start(out=xt, in_=x[r0 : r0 + 128, :])
                stats = pst.tile([128, 2, 6], f32, tag="stats")
                nc.vector.bn_stats(out=stats[:, 0, :], in_=xt[:, 0:512])
                nc.vector.bn_stats(out=stats[:, 1, :], in_=xt[:, 512:1024])
                nc.vector.bn_aggr(out=mv[:, tt, :], in_=stats)

            xTb = pxw.tile([128, NCD, N], bf16, tag="xTb")
            for c in range(NCD):
                nc.scalar.dma_start(out=xTb[:, c, :], in_=xT[c * 128 : (c + 1) * 128, :])

            wk_sb = pw.tile([128, NCD, INNER], bf16, tag="W")
            for c in range(NCD):
                nc.gpsimd.dma_start(
                    out=wk_sb[:, c, :], in_=Wk[c * 128 : (c + 1) * 128, :]
                )
            wq_sb = pw.tile([128, NCD, INNER], bf16, tag="W")
            for c in range(NCD):
                nc.gpsimd.dma_start(
                    out=wq_sb[:, c, :], in_=Wq[c * 128 : (c + 1) * 128, :]
                )
            # rstd (batched): exp(-0.5*ln(var+eps))
            nc.scalar.activation(out=rst, in_=mv[:, :, 1], func=AF.Ln, bias=eps_t)
            nc.scalar.activation(out=rst, in_=rst, func=AF.Exp, scale=-0.5)
            # mu -> [1, N] row via DRAM round trip
            mub = pnrm.tile([128, NT], bf16, tag="mub")
            nc.vector.tensor_copy(out=mub, in_=mv[:, :, 0])
            nc.sync.dma_start(out=mu_d.ap().rearrange("t p -> p t"), in_=mub)
            nc.sync.dma_start(
                out=mu1[0:1, :],
                in_=mu_d.ap().rearrange("t p -> (t p)").partition_broadcast(1),
            )

            def proj_norm_tiles(w_sb, wbb_sb, bb_b, nd, ntiles):
                """k/q projection + l2norm scale -> nd DRAM (bf16).

                Squares/reduces run on DVE; the rsqrt Ln/Exp runs batched per
                8-tile sub-batch so the ACT table set switches at most twice
                per sub-batch (ln and exp live in different 'home' sets).
                """
                for half in range(2):
                    cs = half * 512
                    for b0 in range(0, ntiles, 8):
                        nb = min(8, ntiles - b0)
                        s2b = pnrm.tile([128, 8, 8], f32, tag="s2b")
                        kcbs = []
                        for j in range(nb):
                            tt = b0 + j
                            r0 = tt * 128
                            kp = ppj.tile([128, 512], f32, tag="pj")
                            for c in range(NCD):
                                nc.tensor.matmul(
                                    kp,
                                    lhsT=xTb[:, c, r0 : r0 + 128],
                                    rhs=w_sb[:, c, cs : cs + 512],
                                    start=(c == 0),
                                    stop=False,
                                )
                            nc.tensor.matmul(
                                kp,
                                lhsT=mu1[:, r0 : r0 + 128],
                                rhs=wbb_sb[:, cs : cs + 512],
                                start=False,
                                stop=True,
                            )
                            # copy to bf16 (releases PSUM); square+reduce on DVE
                            kcb = pstash.tile([128, 512], bf16, tag=f"kcb{j}")
                            if beta_zero:
                                nc.vector.tensor_copy(out=kcb, in_=kp)
                            else:
                                # general: k = rstd*(kp) + beta@W (bcast row)
                                kf = pnrm.tile([128, 512], f32, tag="kf")
                                nc.vector.tensor_scalar_mul(
                                    out=kf, in0=kp, scalar1=rst[:, tt : tt + 1]
                                )
                                nc.vector.tensor_tensor(
                                    out=kcb, in0=kf, in1=bb_b[:, cs : cs + 512],
                                    op=OP.add,
                                )
                            kcbs.append(kcb)
                            sq = pnrm.tile([128, 512], bf16, tag="sq")
                            nc.vector.tensor_tensor(
                                out=sq, in0=kcb, in1=kcb, op=OP.mult
                            )
                            nc.vector.tensor_reduce(
                                out=s2b[:, j, :],
                                in_=sq.rearrange("p (g d) -> p g d", g=8),
                                axis=AX.X,
                                op=OP.add,
                            )
                        s2f = s2b.rearrange("p a b -> p (a b)")
                        nc.scalar.activation(out=s2f, in_=s2f, func=AF.Ln, bias=eps30)
                        nc.scalar.activation(out=s2f, in_=s2f, func=AF.Exp, scale=-0.5)
                        nc.vector.tensor_scalar_min(out=s2f, in0=s2f, scalar1=1e12)
                        for j in range(nb):
                            tt = b0 + j
                            r0 = tt * 128
                            rex = pnrm.tile([128, 8, DH], bf16, tag="rex")
                            nc.vector.tensor_copy(
                                out=rex, in_=s2b[:, j, :].broadcast_to([128, 8, DH])
                            )
                            kn = pkn.tile([128, 512], bf16, tag="kn")
                            nc.vector.tensor_tensor(
                                out=kn,
                                in0=kcbs[j],
                                in1=rex.rearrange("p g d -> p (g d)"),
                                op=OP.mult,
                            )
                            nc.sync.dma_start(
                                out=nd[r0 : r0 + 128, cs : cs + 512], in_=kn
                            )

            proj_norm_tiles(wk_sb, wbbk_sb, bbk_b, kn_d, NT)
            # kT transposes (scalar HWDGE ring) + qs*ks per-partition fold
            for p in range(HP):
                nc.scalar.dma_start(
                    out=kT[:, p, :], in_=kn_d[:, p * 128 : (p + 1) * 128],
                    transpose=True,
                )
                if not qk_ones:
                    nc.vector.tensor_scalar_mul(
                        out=kT[:, p, :], in0=kT[:, p, :], scalar1=qk_sb
                    )

            proj_norm_tiles(wq_sb, wbbq_sb, bbq_b, qn_d, NTQ)
            for p in range(HP):
                nc.scalar.dma_start(
                    out=qT[:, p, :], in_=qn_d[:, p * 128 : (p + 1) * 128],
                    transpose=True,
                )

            # ---- v projection -> V' natural layout
            wv_sb = pw.tile([128, NCD, INNER], bf16, tag="W")
            for c in range(NCD):
                nc.gpsimd.dma_start(
                    out=wv_sb[:, c, :], in_=Wv[c * 128 : (c + 1) * 128, :]
                )
            for half in range(2):
                cs = half * 512
                for tt in range(NT):
                    r0 = tt * 128
                    vp = ppj.tile([128, 512], f32, tag="pj")
                    for c in range(NCD):
                        nc.tensor.matmul(
                            vp,
                            lhsT=xTb[:, c, r0 : r0 + 128],
                            rhs=wv_sb[:, c, cs : cs + 512],
                            start=(c == 0),
                            stop=False,
                        )
                    nc.tensor.matmul(
                        vp,
                        lhsT=mu1[:, r0 : r0 + 128],
                        rhs=wbbv_sb[:, cs : cs + 512],
                        start=False,
                        stop=True,
                    )
                    if beta_zero:
                        nc.vector.tensor_scalar_mul(
                            out=vsb[:, tt, half * 8 : (half + 1) * 8, 0:DH],
                            in0=vp.rearrange("p (g d) -> p g d", g=8),
                            scalar1=rst[:, tt : tt + 1],
                        )
                    else:
                        vf = pnrm.tile([128, 512], f32, tag="kf")
                        nc.vector.tensor_scalar_mul(
                            out=vf, in0=vp, scalar1=rst[:, tt : tt + 1]
                        )
                        nc.vector.tensor_tensor(
                            out=vsb[:, tt, half * 8 : (half + 1) * 8, 0:DH],
                            in0=vf.rearrange("p (g d) -> p g d", g=8),
                            in1=bbv_b[:, cs : cs + 512].rearrange(
                                "p (g d) -> p g d", g=8
                            ),
                            op=OP.add,
                        )

        # ---------------- attention + output projection ----------------
        QB = NQ // 512  # 2 query blocks of 512
        with (
            tc.tile_pool(name="pwo", bufs=1) as pwo,
            tc.tile_pool(name="pstt", bufs=2, space="PSUM") as pstt,
            tc.tile_pool(name="pot", bufs=3, space="PSUM") as pot,
            tc.tile_pool(name="po", bufs=1, space="PSUM") as po,
            tc.tile_pool(name="ppt", bufs=3) as ppt,
            tc.tile_pool(name="pptn", bufs=2) as pptn,
            tc.tile_pool(name="potf", bufs=3) as potf,
            tc.tile_pool(name="prec", bufs=2) as prec,
            tc.tile_pool(name="pbsc", bufs=2) as pbsc,
            tc.tile_pool(name="pob", bufs=3) as pob,
        ):
            wo_sb = pwo.tile([128, NCD, INNER], bf16, tag="Wo")
            for c in range(NCD):
                nc.gpsimd.dma_start(
                    out=wo_sb[:, c, :], in_=Wo[c * 128 : (c + 1) * 128, :]
                )

            def oproj_group(half, tt):
                r0 = tt * 128
                cs = half * 512
                op_ = po.tile([128, 512], f32, tag="op")
                for c in range(NCD):
                    nc.tensor.matmul(
                        op_,
                        lhsT=AT[:, c, r0 : r0 + 128],
                        rhs=wo_sb[:, c, cs : cs + 512],
                        start=(c == 0),
                        stop=(c == NCD - 1),
                    )
                ob = pob.tile([128, 512], f32, tag="ob")
                nc.vector.tensor_copy(out=ob, in_=op_)
                nc.sync.dma_start(out=out[r0 : r0 + 128, cs : cs + 512], in_=ob)

            for hp in range(HP):
                hA, hB = 2 * hp, 2 * hp + 1
                for qb in range(QB):
                    q0 = qb * 512
                    last_pair = hp == HP - 1 and qb == QB - 1
                    # null scores for both heads
                    st_n = pstt.tile([128, 2, 512], f32, tag="st")
                    null_ps = st_n[0:2, 0, :]
                    nc.tensor.matmul(
                        null_ps,
                        lhsT=nkn_sb[:, hA : hA + 2],
                        rhs=qT[:, hp, q0 : q0 + 512],
                        start=True,
                        stop=True,
                    )
                    pTn = pptn.tile([2, 512], bf16)
                    nc.scalar.activation(out=pTn, in_=null_ps, func=AF.Exp, scale=SCALE)

                    otA = pot.tile([DH + 1, 512], f32, tag="ot")
                    otB = pot.tile([DH + 1, 512], f32, tag="ot")

                    for c in range(NT):
                        st = pstt.tile([128, 2, 512], f32, tag="st")
                        for si, rh in ((0, 0), (1, 1)):
                            nc.tensor.matmul(
                                st[:, si, :],
                                lhsT=kT[
                                    rh * DH : (rh + 1) * DH, hp, c * 128 : (c + 1) * 128
                                ],
                                rhs=qT[rh * DH : (rh + 1) * DH, hp, q0 : q0 + 512],
                                start=True,
                                stop=True,
                                tile_position=(rh * DH, 0),
                            )
                        pt = ppt.tile([128, 2, 512], bf16)
                        nc.scalar.activation(out=pt, in_=st, func=AF.Exp, scale=SCALE)
                        for ot, si, h in ((otA, 0, hA), (otB, 1, hB)):
                            nc.tensor.matmul(
                                ot,
                                lhsT=vsb[:, c, h, :],
                                rhs=pt[:, si, :],
                                start=(c == 0),
                                stop=False,
                            )
                        # interleave o-proj for the first query block into the
                        # last attention pair's PE slack
                        if last_pair and c % 2 == 1:
                            g = c // 2
                            oproj_group(g // 4, g % 4)
                    nc.tensor.matmul(
                        otA, lhsT=nv_sb[:, hA, :], rhs=pTn, start=False, stop=True
                    )
                    nc.tensor.matmul(
                        otB, lhsT=nv_sb[:, hB, :], rhs=pTn, start=False, stop=True
                    )
                    # drain PSUM fast (frees the ot slot for the next pair),
                    # then run the slow denominator broadcast chain from SBUF
                    for h, ot in ((hA, otA), (hB, otB)):
                        otf = potf.tile([DH + 1, 512], f32, tag="otf")
                        nc.vector.tensor_copy(out=otf, in_=ot)
                        den_s = prec.tile([1, 512], f32, tag="dens")
                        nc.vector.tensor_copy(out=den_s, in_=otf[DH : DH + 1, :])
                        nc.sync.dma_start(
                            out=den_d[h, qb]
                            .rearrange("a b -> (a b)")
                            .partition_broadcast(1),
                            in_=den_s,
                        )
                        dd = prec.tile([DH, 8], f32, tag="dd")
                        nc.sync.dma_start(out=dd, in_=den_d[h, qb])
                        rr = prec.tile([DH, 8], f32, tag="rr")
                        nc.vector.reciprocal(rr, dd)
                        nc.sync.dma_start(out=rcp_d[h, qb], in_=rr)
                        rcs = pbsc.tile([DH, 512], f32, tag="bcs")
                        nc.sync.dma_start(
                            out=rcs,
                            in_=rcp_d[h, qb]
                            .rearrange("a b -> (a b)")
                            .partition_broadcast(DH),
                        )
                        po_ = (h % 2) * DH
                        nc.vector.tensor_tensor(
                            out=AT[po_ : po_ + DH, hp, q0 : q0 + 512],
                            in0=otf[0:DH, :],
                            in1=rcs,
                            op=OP.mult,
                        )

            # remaining o-proj groups (query block 1 tokens)
            for half in range(2):
                for tt in range(4, NTQ):
                    oproj_group(half, tt)

    nc.compile()
    return nc


def _get_program(beta_zero: bool = True, qk_ones: bool = True):
    key = ("nc", beta_zero, qk_ones)
    if key not in _CACHE:
        _CACHE[key] = _build_program(beta_zero, qk_ones)
    return _CACHE[key]


def _prep(inputs) -> tuple[list[dict], bool]:
    """Host-side prep: shard + precompute per-core parameter maps."""
    import ml_dtypes

    bf16 = ml_dtypes.bfloat16

    x = np.asarray(inputs["x"], dtype=np.float32)
    gamma = np.asarray(inputs["gamma"], dtype=np.float64)
    beta = np.asarray(inputs["beta"], dtype=np.float64)
    null_kv = np.asarray(inputs["null_kv"], dtype=np.float64)
    Wq = np.asarray(inputs["Wq"], dtype=np.float64)
    Wkv = np.asarray(inputs["Wkv"], dtype=np.float64)
    qs = np.asarray(inputs["q_scale"], dtype=np.float64)
    ks = np.asarray(inputs["k_scale"], dtype=np.float64)
    Wo = np.asarray(inputs["Wo"], dtype=np.float64)

    beta_zero = not np.any(beta)

    Wk = Wkv[:, :INNER]
    Wv = Wkv[:, INNER:]

    def prep_w(W):
        Wg = (gamma[:, None] * W).astype(bf16)
        wbar = Wg.astype(np.float64).sum(axis=0)
        wbb = np.ascontiguousarray((-wbar).astype(bf16).reshape(1, INNER))
        bb = np.ascontiguousarray((beta @ W).astype(np.float32))
        return np.ascontiguousarray(Wg), wbb, bb

    Wk_b, wbbk, bbk = prep_w(Wk)
    Wq_b, wbbq, bbq = prep_w(Wq)
    Wv_b, wbbv, bbv = prep_w(Wv)
    Wo_b = np.ascontiguousarray(Wo.astype(bf16))

    # null-kv prep (float64): nkn = l2norm(nk) * (qs*ks), block-diagonal
    nk = null_kv[0, :, 0, :]  # [H, DH]
    nv = null_kv[1, :, 0, :]
    nrm = np.sqrt((nk * nk).sum(-1, keepdims=True))
    nkn = nk / np.maximum(nrm, 1e-12) * (qs * ks)[None, :]
    nkn_bd = np.zeros((128, HEADS), dtype=np.float64)
    for h in range(HEADS):
        if h % 2 == 0:
            nkn_bd[0:DH, h] = nkn[h]
        else:
            nkn_bd[DH:128, h] = nkn[h]
    nv_bd2 = np.zeros((2, HEADS, DH + 1), dtype=np.float64)
    for h in range(HEADS):
        nv_bd2[h % 2, h, 0:DH] = nv[h]
        nv_bd2[h % 2, h, DH] = 1.0
    nkn_bd = nkn_bd.astype(bf16)
    nv_bd2 = nv_bd2.astype(bf16)

    qkcol = np.tile((qs * ks).astype(np.float32), 2).reshape(128, 1)
    qkcol = np.ascontiguousarray(qkcol)
    qk_ones = bool(np.all(qs * ks == 1.0))

    in_maps = []
    for b in range(B):
        for hi in range(2):
            xb = x[b]
            if hi == 1:
                xb = np.concatenate([xb[NQ:], xb[:NQ]], axis=0)
            xb = np.ascontiguousarray(xb)
            xTb = np.ascontiguousarray(xb.T.astype(bf16))
            m = {
                "x": xb,
                "xT": xTb,
                "Wk": Wk_b,
                "Wq": Wq_b,
                "Wv": Wv_b,
                "Wo": Wo_b,
                "wbbk": wbbk,
                "wbbq": wbbq,
                "wbbv": wbbv,
                "nkn_bd": nkn_bd,
                "nv_bd2": nv_bd2,
                "qkcol": qkcol,
            }
            if not beta_zero:
                m.update({"bbk": bbk, "bbq": bbq, "bbv": bbv})
            in_maps.append(m)
    return in_maps, (beta_zero, qk_ones)


def kernel(**inputs) -> np.ndarray:
    from concourse.bass_utils import run_bass_kernel_spmd

    in_maps, (beta_zero, qk_ones) = _prep(inputs)
    nc = _get_program(beta_zero=beta_zero, qk_ones=qk_ones)

    res = run_bass_kernel_spmd(nc, in_maps, list(range(8)))

    full = np.empty((B, N, DIM), dtype=np.float32)
    for c in range(8):
        b, hi = divmod(c, 2)
        full[b, hi * NQ : (hi + 1) * NQ] = res.results[c]["out"]
    return full


# revision 22
# speedup vs baseline: 1.3695x; 1.1989x over previous
"""Trainium2 Bass kernel for nn_Attention (LayerNorm + L2-normalized-QK attention
with null-kv slot + output projection), SPMD across 8 NeuronCores.

Sharding: core c = (batch b = c//2, query-half hi = c%2). Each core computes the
full kv (2048 tokens) of its batch and attention outputs for its 1024-query
half. Softmax over kv is permutation invariant, so for hi=1 we feed x with the
two sequence halves swapped — every core runs the identical SPMD program with
its queries in rows 0:1024. The final output is a pure concatenation of the
per-core results (no collectives).

Key structural ideas (v1 rewrite):
  - LayerNorm is folded into the projections: project raw x^T (host passes a
    pre-transposed bf16 copy), then subtract the rank-1 correction
    mu[t] * colsum(gamma*W) via one extra K=2 matmul accumulated into the same
    PSUM tile (row0 = mu, row1 = ones for the beta term). The per-token rstd
    cancels inside the q/k L2 norms and is applied to v as a per-partition
    scalar. This removes the serial LN -> transpose -> proj phase entirely.
  - S is computed transposed (S^T [kv, q]); q,k are L2-normalized so
    |8*q.k| <= 8 and exp() cannot overflow (no row-max pass).
  - The two heads of a pair run their S matmuls concurrently via row-group
    tiling (K=64 each at tile_position (0,0)/(64,0)).
  - PV uses V' = [V | 1] (M=65) so the softmax denominator falls out of the
    same matmul chain, and the output lands directly in A^T layout for the
    output projection.
  - rsqrt = exp(-0.5*ln(x)): the only ACT table set used in the whole program
    is natural_log_exp_and_others (square/ln/exp all live there -> one load).
  - q_scale*k_scale is folded into kT as a per-partition scale post-transpose.
  - Weights are pre-folded with gamma, cast to bf16, and reordered on host;
    null-kv tensors are fully precomputed on host in float64.
"""

import numpy as np

B = 4
N = 2048
DIM = 1024
HEADS = 16
DH = 64
INNER = HEADS * DH
NQ = 1024  # queries per core
SCALE = 8.0
LN_EPS = 1e-5

_CACHE = {}


def _build_program(beta_zero: bool, qk_ones: bool):
    from contextlib import ExitStack

    import concourse.bacc as bacc
    import concourse.tile as tile
    from concourse import mybir

    f32 = mybir.dt.float32
    bf16 = mybir.dt.bfloat16
    AF = mybir.ActivationFunctionType
    OP = mybir.AluOpType
    AX = mybir.AxisListType

    NT = N // 128          # 16 token tiles
    NTQ = NQ // 128        # 8 query token tiles
    NCD = DIM // 128       # 8 dim chunks
    HP = HEADS // 2        # 8 head pairs

    nc = bacc.Bacc("TRN2", target_bir_lowering=False, debug=False)

    x = nc.declare_dram_parameter("x", [N, DIM], f32, isOutput=False)
    xT = nc.declare_dram_parameter("xT", [DIM, N], bf16, isOutput=False)
    Wk = nc.declare_dram_parameter("Wk", [DIM, INNER], bf16, isOutput=False)
    Wq = nc.declare_dram_parameter("Wq", [DIM, INNER], bf16, isOutput=False)
    Wv = nc.declare_dram_parameter("Wv", [DIM, INNER], bf16, isOutput=False)
    Wo = nc.declare_dram_parameter("Wo", [INNER, DIM], bf16, isOutput=False)
    # correction row: -colsum(gamma*W) (computed from the bf16 weights)
    wbbk = nc.declare_dram_parameter("wbbk", [1, INNER], bf16, isOutput=False)
    wbbq = nc.declare_dram_parameter("wbbq", [1, INNER], bf16, isOutput=False)
    wbbv = nc.declare_dram_parameter("wbbv", [1, INNER], bf16, isOutput=False)
    if not beta_zero:
        bbk = nc.declare_dram_parameter("bbk", [INNER], f32, isOutput=False)
        bbq = nc.declare_dram_parameter("bbq", [INNER], f32, isOutput=False)
        bbv = nc.declare_dram_parameter("bbv", [INNER], f32, isOutput=False)
    nkn_p = nc.declare_dram_parameter("nkn_bd", [128, HEADS], bf16, isOutput=False)
    nv_p = nc.declare_dram_parameter("nv_bd2", [2, HEADS, DH + 1], bf16, isOutput=False)
    # per-partition qs*ks pattern for kT scaling ([d0..63, d0..63])
    qkc = nc.declare_dram_parameter("qkcol", [128, 1], f32, isOutput=False)
    out = nc.declare_dram_parameter("out", [NQ, DIM], f32, isOutput=True)

    # internal DRAM
    kn_d = nc.dram_tensor("kn_d", [N, INNER], bf16)
    qn_d = nc.dram_tensor("qn_d", [NQ, INNER], bf16)
    mu_d = nc.dram_tensor("mu_d", [NT, 128], bf16)
    rcp_d = nc.dram_tensor("rcp_d", [HEADS, 2, DH, 8], f32)
    den_d = nc.dram_tensor("den_d", [HEADS, 2, DH, 8], f32)

    with tile.TileContext(nc) as tc, ExitStack() as ctx:
        persist = ctx.enter_context(tc.tile_pool(name="persist", bufs=1))

        # ---------------- persistent SBUF ----------------
        kT = persist.tile([128, HP, N], bf16, tag="kT")
        qT = persist.tile([128, HP, NQ], bf16, tag="qT")
        vsb = persist.tile([128, NT, HEADS, DH + 1], bf16, tag="v")
        AT = persist.tile([128, NCD, NQ], bf16, tag="AT")

        nkn_sb = persist.tile([128, HEADS], bf16)
        nc.gpsimd.dma_start(out=nkn_sb, in_=nkn_p[:, :])
        nv_sb = persist.tile([2, HEADS, DH + 1], bf16)
        nc.gpsimd.dma_start(out=nv_sb, in_=nv_p[:, :, :])
        wbbk_sb = persist.tile([1, INNER], bf16)
        nc.gpsimd.dma_start(out=wbbk_sb, in_=wbbk[:, :])
        wbbq_sb = persist.tile([1, INNER], bf16)
        nc.gpsimd.dma_start(out=wbbq_sb, in_=wbbq[:, :])
        wbbv_sb = persist.tile([1, INNER], bf16)
        nc.gpsimd.dma_start(out=wbbv_sb, in_=wbbv[:, :])
        qk_sb = persist.tile([128, 1], f32)
        nc.gpsimd.dma_start(out=qk_sb, in_=qkc[:, :])
        if not beta_zero:
            bbk_b = persist.tile([128, INNER], f32)
            nc.gpsimd.dma_start(out=bbk_b, in_=bbk.ap().partition_broadcast(128))
            bbq_b = persist.tile([128, INNER], f32)
            nc.gpsimd.dma_start(out=bbq_b, in_=bbq.ap().partition_broadcast(128))
            bbv_b = persist.tile([128, INNER], f32)
            nc.gpsimd.dma_start(out=bbv_b, in_=bbv.ap().partition_broadcast(128))
        else:
            bbk_b = bbq_b = bbv_b = None

        mu1 = persist.tile([1, N], bf16, tag="mu1")

        mv = persist.tile([128, NT, 2], f32)
        rst = persist.tile([128, NT], f32)
        eps_t = persist.tile([128, 1], f32)
        nc.vector.memset(eps_t, LN_EPS)
        eps30 = persist.tile([128, 1], f32)
        nc.vector.memset(eps30, 1e-30)

        nc.vector.memset(vsb[:, :, :, DH : DH + 1], 1.0)  # ones column of V'

        # ---------------- projections (LN folded in) ----------------
        with (
            tc.tile_pool(name="pxw", bufs=1) as pxw,
            tc.tile_pool(name="pw", bufs=2) as pw,
            tc.tile_pool(name="pxs", bufs=3) as pxs,
            tc.tile_pool(name="pst", bufs=4) as pst,
            tc.tile_pool(name="ppj", bufs=4, space="PSUM") as ppj,
            tc.tile_pool(name="pnrm", bufs=3) as pnrm,
            tc.tile_pool(name="pkn", bufs=3) as pkn,
            tc.tile_pool(name="pstash", bufs=1) as pstash,
        ):
            # stats first: the mu round trip is the critical path to the first
            # finished projection tile. x tiles on the sync ring, xTb on the
            # scalar ring, weights on the gpsimd (SWDGE) ring — all parallel.
            def stats_half(h0):
                for tt in range(h0, h0 + 8):
                    r0 = tt * 128
                    xt = pxs.tile([128, DIM], f32, tag="xt")
                    eng = nc.sync if tt % 2 == 0 else nc.scalar
                    eng.dma_start(out=xt, in_=x[r0 : r0 + 128, :])
                    stats = pst.tile([128, 2, 6], f32, tag="stats")
                    nc.vector.bn_stats(out=stats[:, 0, :], in_=xt[:, 0:512])
                    nc.vector.bn_stats(out=stats[:, 1, :], in_=xt[:, 512:1024])
                    nc.vector.bn_aggr(out=mv[:, tt, :], in_=stats)
                # mu row for this half -> DRAM round trip (dep chain to proj)
                mub = pnrm.tile([128, 8], bf16, tag="mub")
                nc.vector.tensor_copy(out=mub, in_=mv[:, h0 : h0 + 8, 0])
                nc.sync.dma_start(
                    out=mu_d.ap()[h0 : h0 + 8, :].rearrange("t p -> p t"), in_=mub
                )
                nc.sync.dma_start(
                    out=mu1[0:1, h0 * 128 : (h0 + 8) * 128],
                    in_=mu_d.ap()[h0 : h0 + 8, :]
                    .rearrange("t p -> (t p)")
                    .partition_broadcast(1),
                )

            stats_half(0)
            stats_half(8)

            xTb = pxw.tile([128, NCD, N], bf16, tag="xTb")
            for c in range(NCD):
                nc.scalar.dma_start(out=xTb[:, c, :], in_=xT[c * 128 : (c + 1) * 128, :])

            wk_sb = pw.tile([128, NCD, INNER], bf16, tag="W")
            for c in range(NCD):
                nc.gpsimd.dma_start(
                    out=wk_sb[:, c, :], in_=Wk[c * 128 : (c + 1) * 128, :]
                )
            wq_sb = pw.tile([128, NCD, INNER], bf16, tag="W")
            for c in range(NCD):
                nc.gpsimd.dma_start(
                    out=wq_sb[:, c, :], in_=Wq[c * 128 : (c + 1) * 128, :]
                )
            # rstd (batched): exp(-0.5*ln(var+eps))
            nc.scalar.activation(out=rst, in_=mv[:, :, 1], func=AF.Ln, bias=eps_t)
            nc.scalar.activation(out=rst, in_=rst, func=AF.Exp, scale=-0.5)

            def proj_norm_tiles(w_sb, wbb_sb, bb_b, nd, ntiles):
                """k/q projection + l2norm scale -> nd DRAM (bf16).

                Squares/reduces run on DVE; the rsqrt Ln/Exp runs batched per
                8-tile sub-batch so the ACT table set switches at most twice
                per sub-batch (ln and exp live in different 'home' sets).
                """
                for half in range(2):
                    cs = half * 512
                    for b0 in range(0, ntiles, 8):
                        nb = min(8, ntiles - b0)
                        s2b = pnrm.tile([128, 8, 8], f32, tag="s2b")
                        kcbs = []
                        for j in range(nb):
                            tt = b0 + j
                            r0 = tt * 128
                            kp = ppj.tile([128, 512], f32, tag="pj")
                            for c in range(NCD):
                                nc.tensor.matmul(
                                    kp,
                                    lhsT=xTb[:, c, r0 : r0 + 128],
                                    rhs=w_sb[:, c, cs : cs + 512],
                                    start=(c == 0),
                                    stop=False,
                                )
                            nc.tensor.matmul(
                                kp,
                                lhsT=mu1[:, r0 : r0 + 128],
                                rhs=wbb_sb[:, cs : cs + 512],
                                start=False,
                                stop=True,
                            )
                            # copy to bf16 (releases PSUM); square+reduce on DVE
                            kcb = pstash.tile([128, 512], bf16, tag=f"kcb{j}")
                            if beta_zero:
                                nc.vector.tensor_copy(out=kcb, in_=kp)
                            else:
                                # general: k = rstd*(kp) + beta@W (bcast row)
                                kf = pnrm.tile([128, 512], f32, tag="kf")
                                nc.vector.tensor_scalar_mul(
                                    out=kf, in0=kp, scalar1=rst[:, tt : tt + 1]
                                )
                                nc.vector.tensor_tensor(
                                    out=kcb, in0=kf, in1=bb_b[:, cs : cs + 512],
                                    op=OP.add,
                                )
                            kcbs.append(kcb)
                            sq = pnrm.tile([128, 512], bf16, tag="sq")
                            nc.vector.tensor_tensor(
                                out=sq, in0=kcb, in1=kcb, op=OP.mult
                            )
                            nc.vector.tensor_reduce(
                                out=s2b[:, j, :],
                                in_=sq.rearrange("p (g d) -> p g d", g=8),
                                axis=AX.X,
                                op=OP.add,
                            )
                        s2f = s2b.rearrange("p a b -> p (a b)")
                        nc.scalar.activation(out=s2f, in_=s2f, func=AF.Ln, bias=eps30)
                        nc.scalar.activation(out=s2f, in_=s2f, func=AF.Exp, scale=-0.5)
                        nc.vector.tensor_scalar_min(out=s2f, in0=s2f, scalar1=1e12)
                        for j in range(nb):
                            tt = b0 + j
                            r0 = tt * 128
                            rex = pnrm.tile([128, 8, DH], bf16, tag="rex")
                            nc.vector.tensor_copy(
                                out=rex, in_=s2b[:, j, :].broadcast_to([128, 8, DH])
                            )
                            kn = pkn.tile([128, 512], bf16, tag="kn")
                            nc.vector.tensor_tensor(
                                out=kn,
                                in0=kcbs[j],
                                in1=rex.rearrange("p g d -> p (g d)"),
                                op=OP.mult,
                            )
                            nc.sync.dma_start(
                                out=nd[r0 : r0 + 128, cs : cs + 512], in_=kn
                            )

            proj_norm_tiles(wk_sb, wbbk_sb, bbk_b, kn_d, NT)
            # kT transposes (scalar HWDGE ring) + qs*ks per-partition fold
            for p in range(HP):
                nc.scalar.dma_start(
                    out=kT[:, p, :], in_=kn_d[:, p * 128 : (p + 1) * 128],
                    transpose=True,
                )
                if not qk_ones:
                    nc.vector.tensor_scalar_mul(
                        out=kT[:, p, :], in0=kT[:, p, :], scalar1=qk_sb
                    )

            proj_norm_tiles(wq_sb, wbbq_sb, bbq_b, qn_d, NTQ)
            for p in range(HP):
                nc.scalar.dma_start(
                    out=qT[:, p, :], in_=qn_d[:, p * 128 : (p + 1) * 128],
                    transpose=True,
                )

            # ---- v projection -> V' natural layout
            wv_sb = pw.tile([128, NCD, INNER], bf16, tag="W")
            for c in range(NCD):
                nc.gpsimd.dma_start(
                    out=wv_sb[:, c, :], in_=Wv[c * 128 : (c + 1) * 128, :]
                )
            for half in range(2):
                cs = half * 512
                for tt in range(NT):
                    r0 = tt * 128
                    vp = ppj.tile([128, 512], f32, tag="pj")
                    for c in range(NCD):
                        nc.tensor.matmul(
                            vp,
                            lhsT=xTb[:, c, r0 : r0 + 128],
                            rhs=wv_sb[:, c, cs : cs + 512],
                            start=(c == 0),
                            stop=False,
                        )
                    nc.tensor.matmul(
                        vp,
                        lhsT=mu1[:, r0 : r0 + 128],
                        rhs=wbbv_sb[:, cs : cs + 512],
                        start=False,
                        stop=True,
                    )
                    if beta_zero:
                        nc.vector.tensor_scalar_mul(
                            out=vsb[:, tt, half * 8 : (half + 1) * 8, 0:DH],
                            in0=vp.rearrange("p (g d) -> p g d", g=8),
                            scalar1=rst[:, tt : tt + 1],
                        )
                    else:
                        vf = pnrm.tile([128, 512], f32, tag="kf")
                        nc.vector.tensor_scalar_mul(
                            out=vf, in0=vp, scalar1=rst[:, tt : tt + 1]
                        )
                        nc.vector.tensor_tensor(
                            out=vsb[:, tt, half * 8 : (half + 1) * 8, 0:DH],
                            in0=vf.rearrange("p (g d) -> p g d", g=8),
                            in1=bbv_b[:, cs : cs + 512].rearrange(
                                "p (g d) -> p g d", g=8
                            ),
                            op=OP.add,
                        )

        # ---------------- attention + output projection ----------------
        QB = NQ // 512  # 2 query blocks of 512
        with (
            tc.tile_pool(name="pwo", bufs=1) as pwo,
            tc.tile_pool(name="pstt", bufs=2, space="PSUM") as pstt,
            tc.tile_pool(name="pot", bufs=2, space="PSUM") as pot,
            tc.tile_pool(name="po", bufs=2, space="PSUM") as po,
            tc.tile_pool(name="ppt", bufs=3) as ppt,
            tc.tile_pool(name="pptn", bufs=2) as pptn,
            tc.tile_pool(name="potf", bufs=3) as potf,
            tc.tile_pool(name="prec", bufs=2) as prec,
            tc.tile_pool(name="pbsc", bufs=2) as pbsc,
            tc.tile_pool(name="pob", bufs=3) as pob,
        ):
            wo_sb = pwo.tile([128, NCD, INNER], bf16, tag="Wo")
            for c in range(NCD):
                nc.gpsimd.dma_start(
                    out=wo_sb[:, c, :], in_=Wo[c * 128 : (c + 1) * 128, :]
                )

            def oproj_group(half, tt):
                r0 = tt * 128
                cs = half * 512
                op_ = po.tile([128, 512], f32, tag="op")
                for c in range(NCD):
                    nc.tensor.matmul(
                        op_,
                        lhsT=AT[:, c, r0 : r0 + 128],
                        rhs=wo_sb[:, c, cs : cs + 512],
                        start=(c == 0),
                        stop=(c == NCD - 1),
                    )
                ob = pob.tile([128, 512], f32, tag="ob")
                nc.vector.tensor_copy(out=ob, in_=op_)
                nc.sync.dma_start(out=out[r0 : r0 + 128, cs : cs + 512], in_=ob)

            for hp in range(HP):
                hA, hB = 2 * hp, 2 * hp + 1
                for qb in range(QB):
                    q0 = qb * 512
                    last_pair = hp == HP - 1 and qb == QB - 1
                    # null scores for both heads
                    st_n = pstt.tile([128, 2, 512], f32, tag="st")
                    null_ps = st_n[0:2, 0, :]
                    nc.tensor.matmul(
                        null_ps,
                        lhsT=nkn_sb[:, hA : hA + 2],
                        rhs=qT[:, hp, q0 : q0 + 512],
                        start=True,
                        stop=True,
                    )
                    pTn = pptn.tile([2, 512], bf16)
                    nc.scalar.activation(out=pTn, in_=null_ps, func=AF.Exp, scale=SCALE)

                    otA = pot.tile([DH + 1, 512], f32, tag="ot")
                    otB = pot.tile([DH + 1, 512], f32, tag="ot")

                    def s_chunk(c):
                        st = pstt.tile([128, 2, 512], f32, tag="st")
                        for si, rh in ((0, 0), (1, 1)):
                            nc.tensor.matmul(
                                st[:, si, :],
                                lhsT=kT[
                                    rh * DH : (rh + 1) * DH, hp, c * 128 : (c + 1) * 128
                                ],
                                rhs=qT[rh * DH : (rh + 1) * DH, hp, q0 : q0 + 512],
                                start=True,
                                stop=True,
                                tile_position=(rh * DH, 0),
                            )
                        pt = ppt.tile([128, 2, 512], bf16)
                        nc.scalar.activation(out=pt, in_=st, func=AF.Exp, scale=SCALE)
                        return pt

                    # software-pipelined: S(c+1) is emitted before PV(c) so the
                    # PE never sits behind exp(c) in its own queue
                    pt_c = s_chunk(0)
                    for c in range(NT):
                        pt_n = s_chunk(c + 1) if c + 1 < NT else None
                        for ot, si, h in ((otA, 0, hA), (otB, 1, hB)):
                            nc.tensor.matmul(
                                ot,
                                lhsT=vsb[:, c, h, :],
                                rhs=pt_c[:, si, :],
                                start=(c == 0),
                                stop=False,
                            )
                        pt_c = pt_n
                        # interleave o-proj for the first query block into the
                        # last attention pair's PE slack
                        if last_pair and c % 2 == 1:
                            g = c // 2
                            oproj_group(g // 4, g % 4)
                    nc.tensor.matmul(
                        otA, lhsT=nv_sb[:, hA, :], rhs=pTn, start=False, stop=True
                    )
                    nc.tensor.matmul(
                        otB, lhsT=nv_sb[:, hB, :], rhs=pTn, start=False, stop=True
                    )
                    # drain PSUM fast (frees the ot slot for the next pair),
                    # then run the slow denominator broadcast chain from SBUF
                    for h, ot in ((hA, otA), (hB, otB)):
                        otf = potf.tile([DH + 1, 512], f32, tag="otf")
                        nc.vector.tensor_copy(out=otf, in_=ot)
                        den_s = prec.tile([1, 512], f32, tag="dens")
                        nc.vector.tensor_copy(out=den_s, in_=otf[DH : DH + 1, :])
                        nc.sync.dma_start(
                            out=den_d[h, qb]
                            .rearrange("a b -> (a b)")
                            .partition_broadcast(1),
                            in_=den_s,
                        )
                        dd = prec.tile([DH, 8], f32, tag="dd")
                        nc.sync.dma_start(out=dd, in_=den_d[h, qb])
                        rr = prec.tile([DH, 8], f32, tag="rr")
                        nc.vector.reciprocal(rr, dd)
                        nc.sync.dma_start(out=rcp_d[h, qb], in_=rr)
                        rcs = pbsc.tile([DH, 512], f32, tag="bcs")
                        nc.sync.dma_start(
                            out=rcs,
                            in_=rcp_d[h, qb]
                            .rearrange("a b -> (a b)")
                            .partition_broadcast(DH),
                        )
                        po_ = (h % 2) * DH
                        nc.vector.tensor_tensor(
                            out=AT[po_ : po_ + DH, hp, q0 : q0 + 512],
                            in0=otf[0:DH, :],
                            in1=rcs,
                            op=OP.mult,
                        )

            # remaining o-proj groups (query block 1 tokens)
            for half in range(2):
                for tt in range(4, NTQ):
                    oproj_group(half, tt)

    nc.compile()
    return nc


def _get_program(beta_zero: bool = True, qk_ones: bool = True):
    key = ("nc", beta_zero, qk_ones)
    if key not in _CACHE:
        _CACHE[key] = _build_program(beta_zero, qk_ones)
    return _CACHE[key]


def _prep(inputs) -> tuple[list[dict], bool]:
    """Host-side prep: shard + precompute per-core parameter maps."""
    import ml_dtypes

    bf16 = ml_dtypes.bfloat16

    x = np.asarray(inputs["x"], dtype=np.float32)
    gamma = np.asarray(inputs["gamma"], dtype=np.float64)
    beta = np.asarray(inputs["beta"], dtype=np.float64)
    null_kv = np.asarray(inputs["null_kv"], dtype=np.float64)
    Wq = np.asarray(inputs["Wq"], dtype=np.float64)
    Wkv = np.asarray(inputs["Wkv"], dtype=np.float64)
    qs = np.asarray(inputs["q_scale"], dtype=np.float64)
    ks = np.asarray(inputs["k_scale"], dtype=np.float64)
    Wo = np.asarray(inputs["Wo"], dtype=np.float64)

    beta_zero = not np.any(beta)

    Wk = Wkv[:, :INNER]
    Wv = Wkv[:, INNER:]

    def prep_w(W):
        Wg = (gamma[:, None] * W).astype(bf16)
        wbar = Wg.astype(np.float64).sum(axis=0)
        wbb = np.ascontiguousarray((-wbar).astype(bf16).reshape(1, INNER))
        bb = np.ascontiguousarray((beta @ W).astype(np.float32))
        return np.ascontiguousarray(Wg), wbb, bb

    Wk_b, wbbk, bbk = prep_w(Wk)
    Wq_b, wbbq, bbq = prep_w(Wq)
    Wv_b, wbbv, bbv = prep_w(Wv)
    Wo_b = np.ascontiguousarray(Wo.astype(bf16))

    # null-kv prep (float64): nkn = l2norm(nk) * (qs*ks), block-diagonal
    nk = null_kv[0, :, 0, :]  # [H, DH]
    nv = null_kv[1, :, 0, :]
    nrm = np.sqrt((nk * nk).sum(-1, keepdims=True))
    nkn = nk / np.maximum(nrm, 1e-12) * (qs * ks)[None, :]
    nkn_bd = np.zeros((128, HEADS), dtype=np.float64)
    for h in range(HEADS):
        if h % 2 == 0:
            nkn_bd[0:DH, h] = nkn[h]
        else:
            nkn_bd[DH:128, h] = nkn[h]
    nv_bd2 = np.zeros((2, HEADS, DH + 1), dtype=np.float64)
    for h in range(HEADS):
        nv_bd2[h % 2, h, 0:DH] = nv[h]
        nv_bd2[h % 2, h, DH] = 1.0
    nkn_bd = nkn_bd.astype(bf16)
    nv_bd2 = nv_bd2.astype(bf16)

    qkcol = np.tile((qs * ks).astype(np.float32), 2).reshape(128, 1)
    qkcol = np.ascontiguousarray(qkcol)
    qk_ones = bool(np.all(qs * ks == 1.0))

    in_maps = []
    for b in range(B):
        for hi in range(2):
            xb = x[b]
            if hi == 1:
                xb = np.concatenate([xb[NQ:], xb[:NQ]], axis=0)
            xb = np.ascontiguousarray(xb)
            xTb = np.ascontiguousarray(xb.T.astype(bf16))
            m = {
                "x": xb,
                "xT": xTb,
                "Wk": Wk_b,
                "Wq": Wq_b,
                "Wv": Wv_b,
                "Wo": Wo_b,
                "wbbk": wbbk,
                "wbbq": wbbq,
                "wbbv": wbbv,
                "nkn_bd": nkn_bd,
                "nv_bd2": nv_bd2,
                "qkcol": qkcol,
            }
            if not beta_zero:
                m.update({"bbk": bbk, "bbq": bbq, "bbv": bbv})
            in_maps.append(m)
    return in_maps, (beta_zero, qk_ones)


def kernel(**inputs) -> np.ndarray:
    from concourse.bass_utils import run_bass_kernel_spmd

    in_maps, (beta_zero, qk_ones) = _prep(inputs)
    nc = _get_program(beta_zero=beta_zero, qk_ones=qk_ones)

    res = run_bass_kernel_spmd(nc, in_maps, list(range(8)))

    full = np.empty((B, N, DIM), dtype=np.float32)
    for c in range(8):
        b, hi = divmod(c, 2)
        full[b, hi * NQ : (hi + 1) * NQ] = res.results[c]["out"]
    return full


# revision 23
# speedup vs baseline: 1.3937x; 1.0177x over previous
"""Trainium2 Bass kernel for nn_Attention (LayerNorm + L2-normalized-QK attention
with null-kv slot + output projection), SPMD across 8 NeuronCores.

Sharding: core c = (batch b = c//2, query-half hi = c%2). Each core computes the
full kv (2048 tokens) of its batch and attention outputs for its 1024-query
half. Softmax over kv is permutation invariant, so for hi=1 we feed x with the
two sequence halves swapped — every core runs the identical SPMD program with
its queries in rows 0:1024. The final output is a pure concatenation of the
per-core results (no collectives).

Key structural ideas (v1 rewrite):
  - LayerNorm is folded into the projections: project raw x^T (host passes a
    pre-transposed bf16 copy), then subtract the rank-1 correction
    mu[t] * colsum(gamma*W) via one extra K=2 matmul accumulated into the same
    PSUM tile (row0 = mu, row1 = ones for the beta term). The per-token rstd
    cancels inside the q/k L2 norms and is applied to v as a per-partition
    scalar. This removes the serial LN -> transpose -> proj phase entirely.
  - S is computed transposed (S^T [kv, q]); q,k are L2-normalized so
    |8*q.k| <= 8 and exp() cannot overflow (no row-max pass).
  - The two heads of a pair run their S matmuls concurrently via row-group
    tiling (K=64 each at tile_position (0,0)/(64,0)).
  - PV uses V' = [V | 1] (M=65) so the softmax denominator falls out of the
    same matmul chain, and the output lands directly in A^T layout for the
    output projection.
  - rsqrt = exp(-0.5*ln(x)): the only ACT table set used in the whole program
    is natural_log_exp_and_others (square/ln/exp all live there -> one load).
  - q_scale*k_scale is folded into kT as a per-partition scale post-transpose.
  - Weights are pre-folded with gamma, cast to bf16, and reordered on host;
    null-kv tensors are fully precomputed on host in float64.
"""

import numpy as np

B = 4
N = 2048
DIM = 1024
HEADS = 16
DH = 64
INNER = HEADS * DH
NQ = 1024  # queries per core
SCALE = 8.0
LN_EPS = 1e-5

_CACHE = {}


def _build_program(beta_zero: bool, qk_ones: bool):
    from contextlib import ExitStack

    import concourse.bacc as bacc
    import concourse.tile as tile
    from concourse import mybir

    f32 = mybir.dt.float32
    bf16 = mybir.dt.bfloat16
    AF = mybir.ActivationFunctionType
    OP = mybir.AluOpType
    AX = mybir.AxisListType

    NT = N // 128          # 16 token tiles
    NTQ = NQ // 128        # 8 query token tiles
    NCD = DIM // 128       # 8 dim chunks
    HP = HEADS // 2        # 8 head pairs

    nc = bacc.Bacc("TRN2", target_bir_lowering=False, debug=False)

    x = nc.declare_dram_parameter("x", [N, DIM], f32, isOutput=False)
    xT = nc.declare_dram_parameter("xT", [DIM, N], bf16, isOutput=False)
    Wk = nc.declare_dram_parameter("Wk", [DIM, INNER], bf16, isOutput=False)
    Wq = nc.declare_dram_parameter("Wq", [DIM, INNER], bf16, isOutput=False)
    Wv = nc.declare_dram_parameter("Wv", [DIM, INNER], bf16, isOutput=False)
    Wo = nc.declare_dram_parameter("Wo", [INNER, DIM], bf16, isOutput=False)
    # correction row: -colsum(gamma*W) (computed from the bf16 weights)
    wbbk = nc.declare_dram_parameter("wbbk", [1, INNER], bf16, isOutput=False)
    wbbq = nc.declare_dram_parameter("wbbq", [1, INNER], bf16, isOutput=False)
    wbbv = nc.declare_dram_parameter("wbbv", [1, INNER], bf16, isOutput=False)
    if not beta_zero:
        bbk = nc.declare_dram_parameter("bbk", [INNER], f32, isOutput=False)
        bbq = nc.declare_dram_parameter("bbq", [INNER], f32, isOutput=False)
        bbv = nc.declare_dram_parameter("bbv", [INNER], f32, isOutput=False)
    nkn_p = nc.declare_dram_parameter("nkn_bd", [128, HEADS], bf16, isOutput=False)
    nv_p = nc.declare_dram_parameter("nv_bd2", [2, HEADS, DH + 1], bf16, isOutput=False)
    # per-partition qs*ks pattern for kT scaling ([d0..63, d0..63])
    qkc = nc.declare_dram_parameter("qkcol", [128, 1], f32, isOutput=False)
    out = nc.declare_dram_parameter("out", [NQ, DIM], f32, isOutput=True)

    # internal DRAM
    kn_d = nc.dram_tensor("kn_d", [N, INNER], bf16)
    qn_d = nc.dram_tensor("qn_d", [NQ, INNER], bf16)
    mu_d = nc.dram_tensor("mu_d", [NT, 128], bf16)
    rcp_d = nc.dram_tensor("rcp_d", [HEADS, 2, DH, 8], f32)
    den_d = nc.dram_tensor("den_d", [HEADS, 2, DH, 8], f32)

    with tile.TileContext(nc) as tc, ExitStack() as ctx:
        persist = ctx.enter_context(tc.tile_pool(name="persist", bufs=1))

        # ---------------- persistent SBUF ----------------
        kT = persist.tile([128, HP, N], bf16, tag="kT")
        qT = persist.tile([128, HP, NQ], bf16, tag="qT")
        vsb = persist.tile([128, NT, HEADS, DH + 1], bf16, tag="v")
        AT = persist.tile([128, NCD, NQ], bf16, tag="AT")

        nkn_sb = persist.tile([128, HEADS], bf16)
        nc.gpsimd.dma_start(out=nkn_sb, in_=nkn_p[:, :])
        nv_sb = persist.tile([2, HEADS, DH + 1], bf16)
        nc.gpsimd.dma_start(out=nv_sb, in_=nv_p[:, :, :])
        wbbk_sb = persist.tile([1, INNER], bf16)
        nc.gpsimd.dma_start(out=wbbk_sb, in_=wbbk[:, :])
        wbbq_sb = persist.tile([1, INNER], bf16)
        nc.gpsimd.dma_start(out=wbbq_sb, in_=wbbq[:, :])
        wbbv_sb = persist.tile([1, INNER], bf16)
        nc.gpsimd.dma_start(out=wbbv_sb, in_=wbbv[:, :])
        qk_sb = persist.tile([128, 1], f32)
        nc.gpsimd.dma_start(out=qk_sb, in_=qkc[:, :])
        if not beta_zero:
            bbk_b = persist.tile([128, INNER], f32)
            nc.gpsimd.dma_start(out=bbk_b, in_=bbk.ap().partition_broadcast(128))
            bbq_b = persist.tile([128, INNER], f32)
            nc.gpsimd.dma_start(out=bbq_b, in_=bbq.ap().partition_broadcast(128))
            bbv_b = persist.tile([128, INNER], f32)
            nc.gpsimd.dma_start(out=bbv_b, in_=bbv.ap().partition_broadcast(128))
        else:
            bbk_b = bbq_b = bbv_b = None

        mu1 = persist.tile([1, N], bf16, tag="mu1")

        mv = persist.tile([128, NT, 2], f32)
        rst = persist.tile([128, NT], f32)
        eps_t = persist.tile([128, 1], f32)
        nc.vector.memset(eps_t, LN_EPS)
        eps30 = persist.tile([128, 1], f32)
        nc.vector.memset(eps30, 1e-30)

        nc.vector.memset(vsb[:, :, :, DH : DH + 1], 1.0)  # ones column of V'

        # ---------------- projections (LN folded in) ----------------
        with (
            tc.tile_pool(name="pxw", bufs=1) as pxw,
            tc.tile_pool(name="pw", bufs=2) as pw,
            tc.tile_pool(name="pxs", bufs=3) as pxs,
            tc.tile_pool(name="pst", bufs=4) as pst,
            tc.tile_pool(name="ppj", bufs=6, space="PSUM") as ppj,
            tc.tile_pool(name="pnrm", bufs=3) as pnrm,
            tc.tile_pool(name="pkn", bufs=3) as pkn,
            tc.tile_pool(name="pstash", bufs=1) as pstash,
        ):
            # stats first: the mu round trip is the critical path to the first
            # finished projection tile. x tiles on the sync ring, xTb on the
            # scalar ring, weights on the gpsimd (SWDGE) ring — all parallel.
            def stats_half(h0):
                for tt in range(h0, h0 + 8):
                    r0 = tt * 128
                    xt = pxs.tile([128, DIM], f32, tag="xt")
                    eng = nc.sync if tt % 2 == 0 else nc.scalar
                    eng.dma_start(out=xt, in_=x[r0 : r0 + 128, :])
                    stats = pst.tile([128, 2, 6], f32, tag="stats")
                    nc.vector.bn_stats(out=stats[:, 0, :], in_=xt[:, 0:512])
                    nc.vector.bn_stats(out=stats[:, 1, :], in_=xt[:, 512:1024])
                    nc.vector.bn_aggr(out=mv[:, tt, :], in_=stats)
                # mu row for this half -> DRAM round trip (dep chain to proj)
                mub = pnrm.tile([128, 8], bf16, tag="mub")
                nc.vector.tensor_copy(out=mub, in_=mv[:, h0 : h0 + 8, 0])
                nc.sync.dma_start(
                    out=mu_d.ap()[h0 : h0 + 8, :].rearrange("t p -> p t"), in_=mub
                )
                nc.sync.dma_start(
                    out=mu1[0:1, h0 * 128 : (h0 + 8) * 128],
                    in_=mu_d.ap()[h0 : h0 + 8, :]
                    .rearrange("t p -> (t p)")
                    .partition_broadcast(1),
                )

            stats_half(0)
            stats_half(8)

            xTb = pxw.tile([128, NCD, N], bf16, tag="xTb")
            for c in range(NCD):
                nc.scalar.dma_start(out=xTb[:, c, :], in_=xT[c * 128 : (c + 1) * 128, :])

            wk_sb = pw.tile([128, NCD, INNER], bf16, tag="W")
            for c in range(NCD):
                nc.gpsimd.dma_start(
                    out=wk_sb[:, c, :], in_=Wk[c * 128 : (c + 1) * 128, :]
                )
            wq_sb = pw.tile([128, NCD, INNER], bf16, tag="W")
            for c in range(NCD):
                nc.gpsimd.dma_start(
                    out=wq_sb[:, c, :], in_=Wq[c * 128 : (c + 1) * 128, :]
                )
            # rstd (batched): exp(-0.5*ln(var+eps))
            nc.scalar.activation(out=rst, in_=mv[:, :, 1], func=AF.Ln, bias=eps_t)
            nc.scalar.activation(out=rst, in_=rst, func=AF.Exp, scale=-0.5)

            def proj_norm_tiles(w_sb, wbb_sb, bb_b, nd, ntiles):
                """k/q projection + l2norm scale -> nd DRAM (bf16).

                Squares/reduces run on DVE; the rsqrt Ln/Exp runs batched per
                8-tile sub-batch so the ACT table set switches at most twice
                per sub-batch (ln and exp live in different 'home' sets).
                """
                for half in range(2):
                    cs = half * 512
                    for b0 in range(0, ntiles, 8):
                        nb = min(8, ntiles - b0)
                        s2b = pnrm.tile([128, 8, 8], f32, tag="s2b")
                        kcbs = []
                        for j in range(nb):
                            tt = b0 + j
                            r0 = tt * 128
                            kp = ppj.tile([128, 512], f32, tag="pj")
                            for c in range(NCD):
                                nc.tensor.matmul(
                                    kp,
                                    lhsT=xTb[:, c, r0 : r0 + 128],
                                    rhs=w_sb[:, c, cs : cs + 512],
                                    start=(c == 0),
                                    stop=False,
                                )
                            nc.tensor.matmul(
                                kp,
                                lhsT=mu1[:, r0 : r0 + 128],
                                rhs=wbb_sb[:, cs : cs + 512],
                                start=False,
                                stop=True,
                            )
                            # copy to bf16 (releases PSUM); square+reduce on DVE
                            kcb = pstash.tile([128, 512], bf16, tag=f"kcb{j}")
                            if beta_zero:
                                nc.vector.tensor_copy(out=kcb, in_=kp)
                            else:
                                # general: k = rstd*(kp) + beta@W (bcast row)
                                kf = pnrm.tile([128, 512], f32, tag="kf")
                                nc.vector.tensor_scalar_mul(
                                    out=kf, in0=kp, scalar1=rst[:, tt : tt + 1]
                                )
                                nc.vector.tensor_tensor(
                                    out=kcb, in0=kf, in1=bb_b[:, cs : cs + 512],
                                    op=OP.add,
                                )
                            kcbs.append(kcb)
                            sq = pnrm.tile([128, 512], bf16, tag="sq")
                            nc.vector.tensor_tensor(
                                out=sq, in0=kcb, in1=kcb, op=OP.mult
                            )
                            nc.vector.tensor_reduce(
                                out=s2b[:, j, :],
                                in_=sq.rearrange("p (g d) -> p g d", g=8),
                                axis=AX.X,
                                op=OP.add,
                            )
                        s2f = s2b.rearrange("p a b -> p (a b)")
                        nc.scalar.activation(out=s2f, in_=s2f, func=AF.Ln, bias=eps30)
                        nc.scalar.activation(out=s2f, in_=s2f, func=AF.Exp, scale=-0.5)
                        nc.vector.tensor_scalar_min(out=s2f, in0=s2f, scalar1=1e12)
                        for j in range(nb):
                            tt = b0 + j
                            r0 = tt * 128
                            kn = pkn.tile([128, 512], bf16, tag="kn")
                            nc.vector.tensor_tensor(
                                out=kn.rearrange("p (g d) -> p g d", g=8),
                                in0=kcbs[j].rearrange("p (g d) -> p g d", g=8),
                                in1=s2b[:, j, :].broadcast_to([128, 8, DH]),
                                op=OP.mult,
                            )
                            nc.sync.dma_start(
                                out=nd[r0 : r0 + 128, cs : cs + 512], in_=kn
                            )

            proj_norm_tiles(wk_sb, wbbk_sb, bbk_b, kn_d, NT)
            # kT transposes (scalar HWDGE ring) + qs*ks per-partition fold
            for p in range(HP):
                nc.scalar.dma_start(
                    out=kT[:, p, :], in_=kn_d[:, p * 128 : (p + 1) * 128],
                    transpose=True,
                )
                if not qk_ones:
                    nc.vector.tensor_scalar_mul(
                        out=kT[:, p, :], in0=kT[:, p, :], scalar1=qk_sb
                    )

            proj_norm_tiles(wq_sb, wbbq_sb, bbq_b, qn_d, NTQ)
            for p in range(HP):
                nc.scalar.dma_start(
                    out=qT[:, p, :], in_=qn_d[:, p * 128 : (p + 1) * 128],
                    transpose=True,
                )

            # ---- v projection -> V' natural layout
            wv_sb = pw.tile([128, NCD, INNER], bf16, tag="W")
            for c in range(NCD):
                nc.gpsimd.dma_start(
                    out=wv_sb[:, c, :], in_=Wv[c * 128 : (c + 1) * 128, :]
                )
            for half in range(2):
                cs = half * 512
                for tt in range(NT):
                    r0 = tt * 128
                    vp = ppj.tile([128, 512], f32, tag="pj")
                    for c in range(NCD):
                        nc.tensor.matmul(
                            vp,
                            lhsT=xTb[:, c, r0 : r0 + 128],
                            rhs=wv_sb[:, c, cs : cs + 512],
                            start=(c == 0),
                            stop=False,
                        )
                    nc.tensor.matmul(
                        vp,
                        lhsT=mu1[:, r0 : r0 + 128],
                        rhs=wbbv_sb[:, cs : cs + 512],
                        start=False,
                        stop=True,
                    )
                    if beta_zero:
                        nc.vector.tensor_scalar_mul(
                            out=vsb[:, tt, half * 8 : (half + 1) * 8, 0:DH],
                            in0=vp.rearrange("p (g d) -> p g d", g=8),
                            scalar1=rst[:, tt : tt + 1],
                        )
                    else:
                        vf = pnrm.tile([128, 512], f32, tag="kf")
                        nc.vector.tensor_scalar_mul(
                            out=vf, in0=vp, scalar1=rst[:, tt : tt + 1]
                        )
                        nc.vector.tensor_tensor(
                            out=vsb[:, tt, half * 8 : (half + 1) * 8, 0:DH],
                            in0=vf.rearrange("p (g d) -> p g d", g=8),
                            in1=bbv_b[:, cs : cs + 512].rearrange(
                                "p (g d) -> p g d", g=8
                            ),
                            op=OP.add,
                        )

        # ---------------- attention + output projection ----------------
        QB = NQ // 512  # 2 query blocks of 512
        with (
            tc.tile_pool(name="pwo", bufs=1) as pwo,
            tc.tile_pool(name="pstt", bufs=2, space="PSUM") as pstt,
            tc.tile_pool(name="pot", bufs=2, space="PSUM") as pot,
            tc.tile_pool(name="po", bufs=2, space="PSUM") as po,
            tc.tile_pool(name="ppt", bufs=3) as ppt,
            tc.tile_pool(name="pptn", bufs=2) as pptn,
            tc.tile_pool(name="potf", bufs=3) as potf,
            tc.tile_pool(name="prec", bufs=2) as prec,
            tc.tile_pool(name="pbsc", bufs=2) as pbsc,
            tc.tile_pool(name="pob", bufs=3) as pob,
        ):
            wo_sb = pwo.tile([128, NCD, INNER], bf16, tag="Wo")
            for c in range(NCD):
                nc.gpsimd.dma_start(
                    out=wo_sb[:, c, :], in_=Wo[c * 128 : (c + 1) * 128, :]
                )

            def oproj_group(half, tt):
                r0 = tt * 128
                cs = half * 512
                op_ = po.tile([128, 512], f32, tag="op")
                for c in range(NCD):
                    nc.tensor.matmul(
                        op_,
                        lhsT=AT[:, c, r0 : r0 + 128],
                        rhs=wo_sb[:, c, cs : cs + 512],
                        start=(c == 0),
                        stop=(c == NCD - 1),
                    )
                ob = pob.tile([128, 512], f32, tag="ob")
                nc.vector.tensor_copy(out=ob, in_=op_)
                nc.sync.dma_start(out=out[r0 : r0 + 128, cs : cs + 512], in_=ob)

            for hp in range(HP):
                hA, hB = 2 * hp, 2 * hp + 1
                # null scores for both heads and both query blocks, one exp
                st_n = pstt.tile([128, 2, 512], f32, tag="st")
                for qb in range(QB):
                    nc.tensor.matmul(
                        st_n[0:2, qb, :],
                        lhsT=nkn_sb[:, hA : hA + 2],
                        rhs=qT[:, hp, qb * 512 : qb * 512 + 512],
                        start=True,
                        stop=True,
                    )
                pTn2 = pptn.tile([2, 2, 512], bf16)
                nc.scalar.activation(out=pTn2, in_=st_n[0:2, :, :], func=AF.Exp, scale=SCALE)
                for qb in range(QB):
                    q0 = qb * 512
                    last_pair = hp == HP - 1 and qb == QB - 1
                    pTn = pTn2[:, qb, :]

                    otA = pot.tile([DH + 1, 512], f32, tag="ot")
                    otB = pot.tile([DH + 1, 512], f32, tag="ot")

                    def s_chunk(c):
                        st = pstt.tile([128, 2, 512], f32, tag="st")
                        for si, rh in ((0, 0), (1, 1)):
                            nc.tensor.matmul(
                                st[:, si, :],
                                lhsT=kT[
                                    rh * DH : (rh + 1) * DH, hp, c * 128 : (c + 1) * 128
                                ],
                                rhs=qT[rh * DH : (rh + 1) * DH, hp, q0 : q0 + 512],
                                start=True,
                                stop=True,
                                tile_position=(rh * DH, 0),
                            )
                        pt = ppt.tile([128, 2, 512], bf16)
                        nc.scalar.activation(out=pt, in_=st, func=AF.Exp, scale=SCALE)
                        return pt

                    # software-pipelined: S(c+1) is emitted before PV(c) so the
                    # PE never sits behind exp(c) in its own queue
                    pt_c = s_chunk(0)
                    for c in range(NT):
                        pt_n = s_chunk(c + 1) if c + 1 < NT else None
                        for ot, si, h in ((otA, 0, hA), (otB, 1, hB)):
                            nc.tensor.matmul(
                                ot,
                                lhsT=vsb[:, c, h, :],
                                rhs=pt_c[:, si, :],
                                start=(c == 0),
                                stop=False,
                            )
                        pt_c = pt_n
                        # interleave o-proj for the first query block into the
                        # last attention pair's PE slack
                        if last_pair and c % 2 == 1:
                            g = c // 2
                            oproj_group(g // 4, g % 4)
                    nc.tensor.matmul(
                        otA, lhsT=nv_sb[:, hA, :], rhs=pTn, start=False, stop=True
                    )
                    nc.tensor.matmul(
                        otB, lhsT=nv_sb[:, hB, :], rhs=pTn, start=False, stop=True
                    )
                    # drain PSUM fast (frees the ot slot for the next pair),
                    # then run the slow denominator broadcast chain from SBUF
                    for h, ot in ((hA, otA), (hB, otB)):
                        otf = potf.tile([DH + 1, 512], f32, tag="otf")
                        nc.vector.tensor_copy(out=otf, in_=ot)
                        nc.sync.dma_start(
                            out=den_d[h, qb]
                            .rearrange("a b -> (a b)")
                            .partition_broadcast(1),
                            in_=otf[DH : DH + 1, :],
                        )
                        dd = prec.tile([DH, 8], f32, tag="dd")
                        nc.sync.dma_start(out=dd, in_=den_d[h, qb])
                        rr = prec.tile([DH, 8], f32, tag="rr")
                        nc.vector.reciprocal(rr, dd)
                        nc.sync.dma_start(out=rcp_d[h, qb], in_=rr)
                        rcs = pbsc.tile([DH, 512], f32, tag="bcs")
                        nc.sync.dma_start(
                            out=rcs,
                            in_=rcp_d[h, qb]
                            .rearrange("a b -> (a b)")
                            .partition_broadcast(DH),
                        )
                        po_ = (h % 2) * DH
                        nc.vector.tensor_tensor(
                            out=AT[po_ : po_ + DH, hp, q0 : q0 + 512],
                            in0=otf[0:DH, :],
                            in1=rcs,
                            op=OP.mult,
                        )

            # remaining o-proj groups (query block 1 tokens)
            for half in range(2):
                for tt in range(4, NTQ):
                    oproj_group(half, tt)

    nc.compile()
    return nc


def _get_program(beta_zero: bool = True, qk_ones: bool = True):
    key = ("nc", beta_zero, qk_ones)
    if key not in _CACHE:
        _CACHE[key] = _build_program(beta_zero, qk_ones)
    return _CACHE[key]


def _prep(inputs) -> tuple[list[dict], bool]:
    """Host-side prep: shard + precompute per-core parameter maps."""
    import ml_dtypes

    bf16 = ml_dtypes.bfloat16

    x = np.asarray(inputs["x"], dtype=np.float32)
    gamma = np.asarray(inputs["gamma"], dtype=np.float64)
    beta = np.asarray(inputs["beta"], dtype=np.float64)
    null_kv = np.asarray(inputs["null_kv"], dtype=np.float64)
    Wq = np.asarray(inputs["Wq"], dtype=np.float64)
    Wkv = np.asarray(inputs["Wkv"], dtype=np.float64)
    qs = np.asarray(inputs["q_scale"], dtype=np.float64)
    ks = np.asarray(inputs["k_scale"], dtype=np.float64)
    Wo = np.asarray(inputs["Wo"], dtype=np.float64)

    beta_zero = not np.any(beta)

    Wk = Wkv[:, :INNER]
    Wv = Wkv[:, INNER:]

    def prep_w(W):
        Wg = (gamma[:, None] * W).astype(bf16)
        wbar = Wg.astype(np.float64).sum(axis=0)
        wbb = np.ascontiguousarray((-wbar).astype(bf16).reshape(1, INNER))
        bb = np.ascontiguousarray((beta @ W).astype(np.float32))
        return np.ascontiguousarray(Wg), wbb, bb

    Wk_b, wbbk, bbk = prep_w(Wk)
    Wq_b, wbbq, bbq = prep_w(Wq)
    Wv_b, wbbv, bbv = prep_w(Wv)
    Wo_b = np.ascontiguousarray(Wo.astype(bf16))

    # null-kv prep (float64): nkn = l2norm(nk) * (qs*ks), block-diagonal
    nk = null_kv[0, :, 0, :]  # [H, DH]
    nv = null_kv[1, :, 0, :]
    nrm = np.sqrt((nk * nk).sum(-1, keepdims=True))
    nkn = nk / np.maximum(nrm, 1e-12) * (qs * ks)[None, :]
    nkn_bd = np.zeros((128, HEADS), dtype=np.float64)
    for h in range(HEADS):
        if h % 2 == 0:
            nkn_bd[0:DH, h] = nkn[h]
        else:
            nkn_bd[DH:128, h] = nkn[h]
    nv_bd2 = np.zeros((2, HEADS, DH + 1), dtype=np.float64)
    for h in range(HEADS):
        nv_bd2[h % 2, h, 0:DH] = nv[h]
        nv_bd2[h % 2, h, DH] = 1.0
    nkn_bd = nkn_bd.astype(bf16)
    nv_bd2 = nv_bd2.astype(bf16)

    qkcol = np.tile((qs * ks).astype(np.float32), 2).reshape(128, 1)
    qkcol = np.ascontiguousarray(qkcol)
    qk_ones = bool(np.all(qs * ks == 1.0))

    in_maps = []
    for b in range(B):
        for hi in range(2):
            xb = x[b]
            if hi == 1:
                xb = np.concatenate([xb[NQ:], xb[:NQ]], axis=0)
            xb = np.ascontiguousarray(xb)
            xTb = np.ascontiguousarray(xb.T.astype(bf16))
            m = {
                "x": xb,
                "xT": xTb,
                "Wk": Wk_b,
                "Wq": Wq_b,
                "Wv": Wv_b,
                "Wo": Wo_b,
                "wbbk": wbbk,
                "wbbq": wbbq,
                "wbbv": wbbv,
                "nkn_bd": nkn_bd,
                "nv_bd2": nv_bd2,
                "qkcol": qkcol,
            }
            if not beta_zero:
                m.update({"bbk": bbk, "bbq": bbq, "bbv": bbv})
            in_maps.append(m)
    return in_maps, (beta_zero, qk_ones)


def kernel(**inputs) -> np.ndarray:
    from concourse.bass_utils import run_bass_kernel_spmd

    in_maps, (beta_zero, qk_ones) = _prep(inputs)
    nc = _get_program(beta_zero=beta_zero, qk_ones=qk_ones)

    res = run_bass_kernel_spmd(nc, in_maps, list(range(8)))

    full = np.empty((B, N, DIM), dtype=np.float32)
    for c in range(8):
        b, hi = divmod(c, 2)
        full[b, hi * NQ : (hi + 1) * NQ] = res.results[c]["out"]
    return full


# revision 25
# speedup vs baseline: 1.4021x; 1.0060x over previous
"""Trainium2 Bass kernel for nn_Attention (LayerNorm + L2-normalized-QK attention
with null-kv slot + output projection), SPMD across 8 NeuronCores.

Sharding: core c = (batch b = c//2, query-half hi = c%2). Each core computes the
full kv (2048 tokens) of its batch and attention outputs for its 1024-query
half. Softmax over kv is permutation invariant, so for hi=1 we feed x with the
two sequence halves swapped — every core runs the identical SPMD program with
its queries in rows 0:1024. The final output is a pure concatenation of the
per-core results (no collectives).

Key structural ideas (v1 rewrite):
  - LayerNorm is folded into the projections: project raw x^T (host passes a
    pre-transposed bf16 copy), then subtract the rank-1 correction
    mu[t] * colsum(gamma*W) via one extra K=2 matmul accumulated into the same
    PSUM tile (row0 = mu, row1 = ones for the beta term). The per-token rstd
    cancels inside the q/k L2 norms and is applied to v as a per-partition
    scalar. This removes the serial LN -> transpose -> proj phase entirely.
  - S is computed transposed (S^T [kv, q]); q,k are L2-normalized so
    |8*q.k| <= 8 and exp() cannot overflow (no row-max pass).
  - The two heads of a pair run their S matmuls concurrently via row-group
    tiling (K=64 each at tile_position (0,0)/(64,0)).
  - PV uses V' = [V | 1] (M=65) so the softmax denominator falls out of the
    same matmul chain, and the output lands directly in A^T layout for the
    output projection.
  - rsqrt = exp(-0.5*ln(x)): the only ACT table set used in the whole program
    is natural_log_exp_and_others (square/ln/exp all live there -> one load).
  - q_scale*k_scale is folded into kT as a per-partition scale post-transpose.
  - Weights are pre-folded with gamma, cast to bf16, and reordered on host;
    null-kv tensors are fully precomputed on host in float64.
"""

import numpy as np

B = 4
N = 2048
DIM = 1024
HEADS = 16
DH = 64
INNER = HEADS * DH
NQ = 1024  # queries per core
SCALE = 8.0
LN_EPS = 1e-5

_CACHE = {}


def _build_program(beta_zero: bool, qk_ones: bool):
    from contextlib import ExitStack

    import concourse.bacc as bacc
    import concourse.tile as tile
    from concourse import mybir

    f32 = mybir.dt.float32
    bf16 = mybir.dt.bfloat16
    AF = mybir.ActivationFunctionType
    OP = mybir.AluOpType
    AX = mybir.AxisListType

    NT = N // 128          # 16 token tiles
    NTQ = NQ // 128        # 8 query token tiles
    NCD = DIM // 128       # 8 dim chunks
    HP = HEADS // 2        # 8 head pairs

    nc = bacc.Bacc("TRN2", target_bir_lowering=False, debug=False)

    x = nc.declare_dram_parameter("x", [N, DIM], f32, isOutput=False)
    xT = nc.declare_dram_parameter("xT", [DIM, N], bf16, isOutput=False)
    Wk = nc.declare_dram_parameter("Wk", [DIM, INNER], bf16, isOutput=False)
    Wq = nc.declare_dram_parameter("Wq", [DIM, INNER], bf16, isOutput=False)
    Wv = nc.declare_dram_parameter("Wv", [DIM, INNER], bf16, isOutput=False)
    Wo = nc.declare_dram_parameter("Wo", [INNER, DIM], bf16, isOutput=False)
    # correction row: -colsum(gamma*W) (computed from the bf16 weights)
    wbbk = nc.declare_dram_parameter("wbbk", [1, INNER], bf16, isOutput=False)
    wbbq = nc.declare_dram_parameter("wbbq", [1, INNER], bf16, isOutput=False)
    wbbv = nc.declare_dram_parameter("wbbv", [1, INNER], bf16, isOutput=False)
    if not beta_zero:
        bbk = nc.declare_dram_parameter("bbk", [INNER], f32, isOutput=False)
        bbq = nc.declare_dram_parameter("bbq", [INNER], f32, isOutput=False)
        bbv = nc.declare_dram_parameter("bbv", [INNER], f32, isOutput=False)
    nkn_p = nc.declare_dram_parameter("nkn_bd", [128, HEADS], bf16, isOutput=False)
    nv_p = nc.declare_dram_parameter("nv_bd2", [2, HEADS, DH + 1], bf16, isOutput=False)
    # per-partition qs*ks pattern for kT scaling ([d0..63, d0..63])
    qkc = nc.declare_dram_parameter("qkcol", [128, 1], f32, isOutput=False)
    out = nc.declare_dram_parameter("out", [NQ, DIM], f32, isOutput=True)

    # internal DRAM
    kn_d = nc.dram_tensor("kn_d", [N, INNER], bf16)
    qn_d = nc.dram_tensor("qn_d", [NQ, INNER], bf16)
    mu_d = nc.dram_tensor("mu_d", [NT, 128], bf16)
    rcp_d = nc.dram_tensor("rcp_d", [HEADS, 2, DH, 8], f32)
    den_d = nc.dram_tensor("den_d", [HEADS, 2, DH, 8], f32)

    with tile.TileContext(nc) as tc, ExitStack() as ctx:
        persist = ctx.enter_context(tc.tile_pool(name="persist", bufs=1))

        # ---------------- persistent SBUF ----------------
        kT = persist.tile([128, HP, N], bf16, tag="kT")
        qT = persist.tile([128, HP, NQ], bf16, tag="qT")
        vsb = persist.tile([128, NT, HEADS, DH + 1], bf16, tag="v")
        AT = persist.tile([128, NCD, NQ], bf16, tag="AT")

        nkn_sb = persist.tile([128, HEADS], bf16)
        nv_sb = persist.tile([2, HEADS, DH + 1], bf16)
        wbbk_sb = persist.tile([1, INNER], bf16)
        wbbq_sb = persist.tile([1, INNER], bf16)
        wbbv_sb = persist.tile([1, INNER], bf16)
        qk_sb = persist.tile([128, 1], f32)
        if not beta_zero:
            bbk_b = persist.tile([128, INNER], f32)
            nc.gpsimd.dma_start(out=bbk_b, in_=bbk.ap().partition_broadcast(128))
            bbq_b = persist.tile([128, INNER], f32)
            nc.gpsimd.dma_start(out=bbq_b, in_=bbq.ap().partition_broadcast(128))
            bbv_b = persist.tile([128, INNER], f32)
            nc.gpsimd.dma_start(out=bbv_b, in_=bbv.ap().partition_broadcast(128))
        else:
            bbk_b = bbq_b = bbv_b = None

        mu1 = persist.tile([1, N], bf16, tag="mu1")

        mv = persist.tile([128, NT, 2], f32)
        rst = persist.tile([128, NT], f32)
        eps_t = persist.tile([128, 1], f32)
        nc.vector.memset(eps_t, LN_EPS)
        eps30 = persist.tile([128, 1], f32)
        nc.vector.memset(eps30, 1e-30)

        nc.vector.memset(vsb[:, :, :, DH : DH + 1], 1.0)  # ones column of V'

        # ---------------- projections (LN folded in) ----------------
        with (
            tc.tile_pool(name="pxw", bufs=1) as pxw,
            tc.tile_pool(name="pw", bufs=2) as pw,
            tc.tile_pool(name="pxs", bufs=4) as pxs,
            tc.tile_pool(name="pst", bufs=4) as pst,
            tc.tile_pool(name="ppj", bufs=6, space="PSUM") as ppj,
            tc.tile_pool(name="pnrm", bufs=3) as pnrm,
            tc.tile_pool(name="pkn", bufs=3) as pkn,
            tc.tile_pool(name="pstash", bufs=1) as pstash,
        ):
            # stats first: the mu round trip is the critical path to the first
            # finished projection tile. x tiles on the sync ring, xTb on the
            # scalar ring, weights on the gpsimd (SWDGE) ring — all parallel.
            def stats_half(h0):
                for tt in range(h0, h0 + 8):
                    r0 = tt * 128
                    xt = pxs.tile([128, DIM], f32, tag="xt")
                    eng = nc.sync if tt % 2 == 0 else nc.scalar
                    eng.dma_start(out=xt, in_=x[r0 : r0 + 128, :])
                    stats = pst.tile([128, 2, 6], f32, tag="stats")
                    nc.vector.bn_stats(out=stats[:, 0, :], in_=xt[:, 0:512])
                    nc.vector.bn_stats(out=stats[:, 1, :], in_=xt[:, 512:1024])
                    nc.vector.bn_aggr(out=mv[:, tt, :], in_=stats)
                # mu row for this half -> DRAM round trip (dep chain to proj)
                mub = pnrm.tile([128, 8], bf16, tag="mub")
                nc.vector.tensor_copy(out=mub, in_=mv[:, h0 : h0 + 8, 0])
                nc.sync.dma_start(
                    out=mu_d.ap()[h0 : h0 + 8, :].rearrange("t p -> p t"), in_=mub
                )
                nc.sync.dma_start(
                    out=mu1[0:1, h0 * 128 : (h0 + 8) * 128],
                    in_=mu_d.ap()[h0 : h0 + 8, :]
                    .rearrange("t p -> (t p)")
                    .partition_broadcast(1),
                )

            stats_half(0)
            stats_half(8)

            xTb = pxw.tile([128, NCD, N], bf16, tag="xTb")
            for c in range(NCD):
                nc.scalar.dma_start(out=xTb[:, c, :], in_=xT[c * 128 : (c + 1) * 128, :])

            wk_sb = pw.tile([128, NCD, INNER], bf16, tag="W")
            for c in range(NCD):
                nc.gpsimd.dma_start(
                    out=wk_sb[:, c, :], in_=Wk[c * 128 : (c + 1) * 128, :]
                )
            wq_sb = pw.tile([128, NCD, INNER], bf16, tag="W")
            for c in range(NCD):
                nc.gpsimd.dma_start(
                    out=wq_sb[:, c, :], in_=Wq[c * 128 : (c + 1) * 128, :]
                )
            nc.gpsimd.dma_start(out=nkn_sb, in_=nkn_p[:, :])
            nc.gpsimd.dma_start(out=nv_sb, in_=nv_p[:, :, :])
            nc.gpsimd.dma_start(out=wbbk_sb, in_=wbbk[:, :])
            nc.gpsimd.dma_start(out=wbbq_sb, in_=wbbq[:, :])
            nc.gpsimd.dma_start(out=wbbv_sb, in_=wbbv[:, :])
            nc.gpsimd.dma_start(out=qk_sb, in_=qkc[:, :])
            # rstd (batched): exp(-0.5*ln(var+eps))
            nc.scalar.activation(out=rst, in_=mv[:, :, 1], func=AF.Ln, bias=eps_t)
            nc.scalar.activation(out=rst, in_=rst, func=AF.Exp, scale=-0.5)

            def proj_norm_tiles(w_sb, wbb_sb, bb_b, nd, ntiles):
                """k/q projection + l2norm scale -> nd DRAM (bf16).

                Squares/reduces run on DVE; the rsqrt Ln/Exp runs batched per
                8-tile sub-batch so the ACT table set switches at most twice
                per sub-batch (ln and exp live in different 'home' sets).
                """
                for half in range(2):
                    cs = half * 512
                    for b0 in range(0, ntiles, 8):
                        nb = min(8, ntiles - b0)
                        s2b = pnrm.tile([128, 8, 8], f32, tag="s2b")
                        kcbs = []
                        for j in range(nb):
                            tt = b0 + j
                            r0 = tt * 128
                            kp = ppj.tile([128, 512], f32, tag="pj")
                            for c in range(NCD):
                                nc.tensor.matmul(
                                    kp,
                                    lhsT=xTb[:, c, r0 : r0 + 128],
                                    rhs=w_sb[:, c, cs : cs + 512],
                                    start=(c == 0),
                                    stop=False,
                                )
                            nc.tensor.matmul(
                                kp,
                                lhsT=mu1[:, r0 : r0 + 128],
                                rhs=wbb_sb[:, cs : cs + 512],
                                start=False,
                                stop=True,
                            )
                            # copy to bf16 (releases PSUM); square+reduce on DVE
                            kcb = pstash.tile([128, 512], bf16, tag=f"kcb{j}")
                            if beta_zero:
                                nc.vector.tensor_copy(out=kcb, in_=kp)
                            else:
                                # general: k = rstd*(kp) + beta@W (bcast row)
                                kf = pnrm.tile([128, 512], f32, tag="kf")
                                nc.vector.tensor_scalar_mul(
                                    out=kf, in0=kp, scalar1=rst[:, tt : tt + 1]
                                )
                                nc.vector.tensor_tensor(
                                    out=kcb, in0=kf, in1=bb_b[:, cs : cs + 512],
                                    op=OP.add,
                                )
                            kcbs.append(kcb)
                            sq = pnrm.tile([128, 512], bf16, tag="sq")
                            nc.vector.tensor_tensor(
                                out=sq, in0=kcb, in1=kcb, op=OP.mult
                            )
                            nc.vector.tensor_reduce(
                                out=s2b[:, j, :],
                                in_=sq.rearrange("p (g d) -> p g d", g=8),
                                axis=AX.X,
                                op=OP.add,
                            )
                        s2f = s2b.rearrange("p a b -> p (a b)")
                        nc.scalar.activation(out=s2f, in_=s2f, func=AF.Ln, bias=eps30)
                        nc.scalar.activation(out=s2f, in_=s2f, func=AF.Exp, scale=-0.5)
                        nc.vector.tensor_scalar_min(out=s2f, in0=s2f, scalar1=1e12)
                        for j in range(nb):
                            tt = b0 + j
                            r0 = tt * 128
                            kn = pkn.tile([128, 512], bf16, tag="kn")
                            nc.vector.tensor_tensor(
                                out=kn.rearrange("p (g d) -> p g d", g=8),
                                in0=kcbs[j].rearrange("p (g d) -> p g d", g=8),
                                in1=s2b[:, j, :].broadcast_to([128, 8, DH]),
                                op=OP.mult,
                            )
                            nc.sync.dma_start(
                                out=nd[r0 : r0 + 128, cs : cs + 512], in_=kn
                            )

            proj_norm_tiles(wk_sb, wbbk_sb, bbk_b, kn_d, NT)
            # kT transposes (scalar HWDGE ring) + qs*ks per-partition fold
            for p in range(HP):
                nc.scalar.dma_start(
                    out=kT[:, p, :], in_=kn_d[:, p * 128 : (p + 1) * 128],
                    transpose=True,
                )
                if not qk_ones:
                    nc.vector.tensor_scalar_mul(
                        out=kT[:, p, :], in0=kT[:, p, :], scalar1=qk_sb
                    )

            proj_norm_tiles(wq_sb, wbbq_sb, bbq_b, qn_d, NTQ)
            for p in range(HP):
                nc.scalar.dma_start(
                    out=qT[:, p, :], in_=qn_d[:, p * 128 : (p + 1) * 128],
                    transpose=True,
                )

            # ---- v projection -> V' natural layout
            wv_sb = pw.tile([128, NCD, INNER], bf16, tag="W")
            for c in range(NCD):
                nc.gpsimd.dma_start(
                    out=wv_sb[:, c, :], in_=Wv[c * 128 : (c + 1) * 128, :]
                )
            for half in range(2):
                cs = half * 512
                for tt in range(NT):
                    r0 = tt * 128
                    vp = ppj.tile([128, 512], f32, tag="pj")
                    for c in range(NCD):
                        nc.tensor.matmul(
                            vp,
                            lhsT=xTb[:, c, r0 : r0 + 128],
                            rhs=wv_sb[:, c, cs : cs + 512],
                            start=(c == 0),
                            stop=False,
                        )
                    nc.tensor.matmul(
                        vp,
                        lhsT=mu1[:, r0 : r0 + 128],
                        rhs=wbbv_sb[:, cs : cs + 512],
                        start=False,
                        stop=True,
                    )
                    if beta_zero:
                        nc.vector.tensor_scalar_mul(
                            out=vsb[:, tt, half * 8 : (half + 1) * 8, 0:DH],
                            in0=vp.rearrange("p (g d) -> p g d", g=8),
                            scalar1=rst[:, tt : tt + 1],
                        )
                    else:
                        vf = pnrm.tile([128, 512], f32, tag="kf")
                        nc.vector.tensor_scalar_mul(
                            out=vf, in0=vp, scalar1=rst[:, tt : tt + 1]
                        )
                        nc.vector.tensor_tensor(
                            out=vsb[:, tt, half * 8 : (half + 1) * 8, 0:DH],
                            in0=vf.rearrange("p (g d) -> p g d", g=8),
                            in1=bbv_b[:, cs : cs + 512].rearrange(
                                "p (g d) -> p g d", g=8
                            ),
                            op=OP.add,
                        )

        # ---------------- attention + output projection ----------------
        QB = NQ // 512  # 2 query blocks of 512
        with (
            tc.tile_pool(name="pwo", bufs=1) as pwo,
            tc.tile_pool(name="pstt", bufs=2, space="PSUM") as pstt,
            tc.tile_pool(name="pot", bufs=2, space="PSUM") as pot,
            tc.tile_pool(name="po", bufs=2, space="PSUM") as po,
            tc.tile_pool(name="ppt", bufs=4) as ppt,
            tc.tile_pool(name="pptn", bufs=2) as pptn,
            tc.tile_pool(name="potf", bufs=3) as potf,
            tc.tile_pool(name="prec", bufs=2) as prec,
            tc.tile_pool(name="pbsc", bufs=2) as pbsc,
            tc.tile_pool(name="pob", bufs=3) as pob,
        ):
            wo_sb = pwo.tile([128, NCD, INNER], bf16, tag="Wo")
            for c in range(NCD):
                nc.gpsimd.dma_start(
                    out=wo_sb[:, c, :], in_=Wo[c * 128 : (c + 1) * 128, :]
                )

            def oproj_group(half, tt):
                r0 = tt * 128
                cs = half * 512
                op_ = po.tile([128, 512], f32, tag="op")
                for c in range(NCD):
                    nc.tensor.matmul(
                        op_,
                        lhsT=AT[:, c, r0 : r0 + 128],
                        rhs=wo_sb[:, c, cs : cs + 512],
                        start=(c == 0),
                        stop=(c == NCD - 1),
                    )
                ob = pob.tile([128, 512], f32, tag="ob")
                nc.vector.tensor_copy(out=ob, in_=op_)
                nc.sync.dma_start(out=out[r0 : r0 + 128, cs : cs + 512], in_=ob)

            for hp in range(HP):
                hA, hB = 2 * hp, 2 * hp + 1
                # null scores for both heads and both query blocks, one exp
                st_n = pstt.tile([128, 2, 512], f32, tag="st")
                for qb in range(QB):
                    nc.tensor.matmul(
                        st_n[0:2, qb, :],
                        lhsT=nkn_sb[:, hA : hA + 2],
                        rhs=qT[:, hp, qb * 512 : qb * 512 + 512],
                        start=True,
                        stop=True,
                    )
                pTn2 = pptn.tile([2, 2, 512], bf16)
                nc.scalar.activation(out=pTn2, in_=st_n[0:2, :, :], func=AF.Exp, scale=SCALE)
                for qb in range(QB):
                    q0 = qb * 512
                    last_pair = hp == HP - 1 and qb == QB - 1
                    pTn = pTn2[:, qb, :]

                    otA = pot.tile([DH + 1, 512], f32, tag="ot")
                    otB = pot.tile([DH + 1, 512], f32, tag="ot")

                    def s_chunk(c):
                        st = pstt.tile([128, 2, 512], f32, tag="st")
                        for si, rh in ((0, 0), (1, 1)):
                            nc.tensor.matmul(
                                st[:, si, :],
                                lhsT=kT[
                                    rh * DH : (rh + 1) * DH, hp, c * 128 : (c + 1) * 128
                                ],
                                rhs=qT[rh * DH : (rh + 1) * DH, hp, q0 : q0 + 512],
                                start=True,
                                stop=True,
                                tile_position=(rh * DH, 0),
                            )
                        pt = ppt.tile([128, 2, 512], bf16)
                        nc.scalar.activation(out=pt, in_=st, func=AF.Exp, scale=SCALE)
                        return pt

                    # software-pipelined: S(c+1) is emitted before PV(c) so the
                    # PE never sits behind exp(c) in its own queue
                    pt_c = s_chunk(0)
                    for c in range(NT):
                        pt_n = s_chunk(c + 1) if c + 1 < NT else None
                        for ot, si, h in ((otA, 0, hA), (otB, 1, hB)):
                            nc.tensor.matmul(
                                ot,
                                lhsT=vsb[:, c, h, :],
                                rhs=pt_c[:, si, :],
                                start=(c == 0),
                                stop=False,
                            )
                        pt_c = pt_n
                        # interleave o-proj for the first query block into the
                        # last attention pair's PE slack
                        if last_pair and c % 2 == 1:
                            g = c // 2
                            oproj_group(g // 4, g % 4)
                    nc.tensor.matmul(
                        otA, lhsT=nv_sb[:, hA, :], rhs=pTn, start=False, stop=True
                    )
                    nc.tensor.matmul(
                        otB, lhsT=nv_sb[:, hB, :], rhs=pTn, start=False, stop=True
                    )
                    # drain PSUM fast (frees the ot slot for the next pair),
                    # then run the slow denominator broadcast chain from SBUF
                    for h, ot in ((hA, otA), (hB, otB)):
                        otf = potf.tile([DH + 1, 512], f32, tag="otf")
                        nc.vector.tensor_copy(out=otf, in_=ot)
                        nc.sync.dma_start(
                            out=den_d[h, qb]
                            .rearrange("a b -> (a b)")
                            .partition_broadcast(1),
                            in_=otf[DH : DH + 1, :],
                        )
                        dd = prec.tile([DH, 8], f32, tag="dd")
                        nc.sync.dma_start(out=dd, in_=den_d[h, qb])
                        rr = prec.tile([DH, 8], f32, tag="rr")
                        nc.vector.reciprocal(rr, dd)
                        nc.sync.dma_start(out=rcp_d[h, qb], in_=rr)
                        rcs = pbsc.tile([DH, 512], f32, tag="bcs")
                        nc.sync.dma_start(
                            out=rcs,
                            in_=rcp_d[h, qb]
                            .rearrange("a b -> (a b)")
                            .partition_broadcast(DH),
                        )
                        po_ = (h % 2) * DH
                        nc.vector.tensor_tensor(
                            out=AT[po_ : po_ + DH, hp, q0 : q0 + 512],
                            in0=otf[0:DH, :],
                            in1=rcs,
                            op=OP.mult,
                        )

            # remaining o-proj groups (query block 1 tokens)
            for half in range(2):
                for tt in range(4, NTQ):
                    oproj_group(half, tt)

    nc.compile()
    return nc


def _get_program(beta_zero: bool = True, qk_ones: bool = True):
    key = ("nc", beta_zero, qk_ones)
    if key not in _CACHE:
        _CACHE[key] = _build_program(beta_zero, qk_ones)
    return _CACHE[key]


def _prep(inputs) -> tuple[list[dict], bool]:
    """Host-side prep: shard + precompute per-core parameter maps."""
    import ml_dtypes

    bf16 = ml_dtypes.bfloat16

    x = np.asarray(inputs["x"], dtype=np.float32)
    gamma = np.asarray(inputs["gamma"], dtype=np.float64)
    beta = np.asarray(inputs["beta"], dtype=np.float64)
    null_kv = np.asarray(inputs["null_kv"], dtype=np.float64)
    Wq = np.asarray(inputs["Wq"], dtype=np.float64)
    Wkv = np.asarray(inputs["Wkv"], dtype=np.float64)
    qs = np.asarray(inputs["q_scale"], dtype=np.float64)
    ks = np.asarray(inputs["k_scale"], dtype=np.float64)
    Wo = np.asarray(inputs["Wo"], dtype=np.float64)

    beta_zero = not np.any(beta)

    Wk = Wkv[:, :INNER]
    Wv = Wkv[:, INNER:]

    def prep_w(W):
        Wg = (gamma[:, None] * W).astype(bf16)
        wbar = Wg.astype(np.float64).sum(axis=0)
        wbb = np.ascontiguousarray((-wbar).astype(bf16).reshape(1, INNER))
        bb = np.ascontiguousarray((beta @ W).astype(np.float32))
        return np.ascontiguousarray(Wg), wbb, bb

    Wk_b, wbbk, bbk = prep_w(Wk)
    Wq_b, wbbq, bbq = prep_w(Wq)
    Wv_b, wbbv, bbv = prep_w(Wv)
    Wo_b = np.ascontiguousarray(Wo.astype(bf16))

    # null-kv prep (float64): nkn = l2norm(nk) * (qs*ks), block-diagonal
    nk = null_kv[0, :, 0, :]  # [H, DH]
    nv = null_kv[1, :, 0, :]
    nrm = np.sqrt((nk * nk).sum(-1, keepdims=True))
    nkn = nk / np.maximum(nrm, 1e-12) * (qs * ks)[None, :]
    nkn_bd = np.zeros((128, HEADS), dtype=np.float64)
    for h in range(HEADS):
        if h % 2 == 0:
            nkn_bd[0:DH, h] = nkn[h]
        else:
            nkn_bd[DH:128, h] = nkn[h]
    nv_bd2 = np.zeros((2, HEADS, DH + 1), dtype=np.float64)
    for h in range(HEADS):
        nv_bd2[h % 2, h, 0:DH] = nv[h]
        nv_bd2[h % 2, h, DH] = 1.0
    nkn_bd = nkn_bd.astype(bf16)
    nv_bd2 = nv_bd2.astype(bf16)

    qkcol = np.tile((qs * ks).astype(np.float32), 2).reshape(128, 1)
    qkcol = np.ascontiguousarray(qkcol)
    qk_ones = bool(np.all(qs * ks == 1.0))

    in_maps = []
    for b in range(B):
        for hi in range(2):
            xb = x[b]
            if hi == 1:
                xb = np.concatenate([xb[NQ:], xb[:NQ]], axis=0)
            xb = np.ascontiguousarray(xb)
            xTb = np.ascontiguousarray(xb.T.astype(bf16))
            m = {
                "x": xb,
                "xT": xTb,
                "Wk": Wk_b,
                "Wq": Wq_b,
                "Wv": Wv_b,
                "Wo": Wo_b,
                "wbbk": wbbk,
                "wbbq": wbbq,
                "wbbv": wbbv,
                "nkn_bd": nkn_bd,
                "nv_bd2": nv_bd2,
                "qkcol": qkcol,
            }
            if not beta_zero:
                m.update({"bbk": bbk, "bbq": bbq, "bbv": bbv})
            in_maps.append(m)
    return in_maps, (beta_zero, qk_ones)


def kernel(**inputs) -> np.ndarray:
    from concourse.bass_utils import run_bass_kernel_spmd

    in_maps, (beta_zero, qk_ones) = _prep(inputs)
    nc = _get_program(beta_zero=beta_zero, qk_ones=qk_ones)

    res = run_bass_kernel_spmd(nc, in_maps, list(range(8)))

    full = np.empty((B, N, DIM), dtype=np.float32)
    for c in range(8):
        b, hi = divmod(c, 2)
        full[b, hi * NQ : (hi + 1) * NQ] = res.results[c]["out"]
    return full
